# revision 9
# baseline (speedup 1.0000x reference)
"""Trainium2 Bass kernel for nn_MixtureOfAdapter (moe_routing), v3.

Math (per token, H=1024, F=256, D=3 domains; grading inputs have
ln_g=1, ln_b=0, b1=0, b2=0, gb=0):
    mu, sd (ddof=1) over H;  s = sd + eps;  xn = (x - mu)/s
    mid_d = relu(W1g_d xn + b1e_d);  a_d = W2_d mid_d + b2_d
    gate_d = sigmoid(gu_d.x + gv_d.a_d + gb_d)
    out = 2x + sum_d gate_d * a_d

Kernel strategy (8 cores, data-parallel over batch B=8):
  - Ship TWO copies of x per core: natural f32 [L,H] (stats via
    bn_stats + final residual) and a host-transposed [H,L] copy in a
    compact dtype (f16, or fp8e4 in fp8 mode) that feeds all matmuls.
    No PE transposes of x, no on-device centering.
  - M1 runs on UNCENTERED xT; centering folds into a rank-1
    correction: out1 = W1q^T xq - colsum(W1q)*mu.  mu itself comes
    from the PE (ones^T @ xT / H), so M1 never waits on the natural-x
    DMA or the stats chain.  relu(out1) = s*mid ("mid_s"), f16.
  - fp8 mode: W1 split hi+lo fp8e4 (noise-free weights), xT single
    fp8; DoubleRow matmuls (0.5 cycles/row, 2 k-chunks/pass) = 4x
    f32r rate.  SC=8 on (W1, gu, gb); relu/sigmoid absorb 1/SC.
  - Gates in [token, domain] layout: pgux[t,d] + gb rank-1; pgv[t,d]
    from mid chunks; z = pgv*r8 + pgux (DVE); gate = sigmoid(z/SC);
    rg = gate*r -> [128,12] -> one PE transpose -> one-hot broadcast
    per (ss,d) -> Act copy to SBUF -> gmid = mid*rg on Pool (f16).
  - M2 (f16) accumulates all domains into one PSUM per (ss, half);
    out = 2x + pout via one DVE scalar_tensor_tensor per half.
  - Emission interleaves next tile's M1 chunks into the gate chain so
    the PE FIFO never drains behind DVE/Act/Pool latency.
"""

import numpy as np

import concourse.bass as bass
import concourse.mybir as mybir
import concourse.tile as tile
from concourse.bass_utils import run_bass_kernel_spmd

B, L, H, F, D = 8, 2048, 1024, 256, 3
EPS = 1e-6
T = 512                 # tokens per macro-tile
NSUB = T // 128         # 4 sub-tiles of 128 tokens
NMT = L // T            # 4 macro-tiles per core
KCH = H // 128          # 8 k-chunks over H
FCH = (D * F) // 128    # 6 chunks over stacked (domain, F)
NCH = H // 512          # 2 output column chunks
DF = D * F

f32 = mybir.dt.float32
f16 = mybir.dt.float16
f8 = mybir.dt.float8e4
AF = mybir.ActivationFunctionType
ALU = mybir.AluOpType
DR = mybir.MatmulPerfMode.DoubleRow

MODE = "fp8"            # "fp8" (DoubleRow M1) or "f16"

# row16 layout (single-partition f16 consts)
RO_ONES = 0             # [0:128] ones
RO_GB = 128             # [128:131] SC*gb_eff
RO_W1S = 131            # [131:899] -SC*colsum(W1g)
RO_B1E = 899            # [899:1667] SC*b1e
RO_END = 1667


def _split_multiwaits(nc):
    """This walrus build allows 1 sync-wait per instruction (2 for
    EventSemaphore); Tile can attach more.  Move extras onto preceding
    same-engine NoOps (engine queues are FIFO, so semantics identical)."""
    for fn in nc.m.functions:
        for bb in fn.blocks:
            new = []
            changed = False
            for inst in bb.instructions:
                si = inst.sync_info
                cap = 2 if isinstance(inst, mybir.InstEventSemaphore) else 1
                if si is not None and len(si.on_wait) > cap:
                    waits = list(si.on_wait)
                    extra, kept = waits[:-cap], waits[-cap:]
                    for j, w in enumerate(extra):
                        new.append(mybir.InstNoOp(
                            name=f"{inst.name}-wsplit{j}",
                            engine=inst.engine,
                            sync_info=mybir.SyncInfo(on_wait=[w], on_update=[]),
                            ins=[], outs=[],
                        ))
                    inst.sync_info = mybir.SyncInfo(
                        on_wait=kept, on_update=list(si.on_update))
                    changed = True
                new.append(inst)
            if changed:
                bb.instructions = new


def _build(has_b1e: bool, has_b2: bool, mode: str = None):
    mode = mode or MODE
    fp8 = mode == "fp8"
    dt1 = f8 if fp8 else f16
    SC = 8.0 if fp8 else 1.0
    ISC = 1.0 / SC

    nc = bass.Bass(target_bir_lowering=False)

    xin = nc.dram_tensor("xin", [L, H], f32, kind="ExternalInput")
    xt = nc.dram_tensor("xt", [H, L], dt1, kind="ExternalInput")
    w1hi = nc.dram_tensor("w1hi", [128, KCH, DF], dt1, kind="ExternalInput")
    if fp8:
        w1lo = nc.dram_tensor("w1lo", [128, KCH, DF], f8, kind="ExternalInput")
    w2t = nc.dram_tensor("w2t", [128, FCH, H], f16, kind="ExternalInput")
    cpack = nc.dram_tensor("cpack", [128, 128], f32, kind="ExternalInput")
    cpk16 = nc.dram_tensor("cpk16", [128, 170], f16, kind="ExternalInput")
    row16 = nc.dram_tensor("row16", [1, RO_END], f16, kind="ExternalInput")
    oneh = nc.dram_tensor("oneh", [12, 12 * 128], f16, kind="ExternalInput")
    if fp8:
        cpk8 = nc.dram_tensor("cpk8", [128, 280], f8, kind="ExternalInput")
    if has_b2:
        b2bc = nc.dram_tensor("b2bc", [128, D, H], f32, kind="ExternalInput")
    out = nc.dram_tensor("out", [L, H], f32, kind="ExternalOutput")

    x_mt = xin.ap().rearrange("(m s p) h -> m p s h", p=128, s=NSUB)
    xt_v = xt.ap().rearrange("(k p) l -> p k l", p=128)
    out_mt = out.ap().rearrange("(m s p) h -> m p s h", p=128, s=NSUB)

    with tile.TileContext(nc) as tc:
        with (
            tc.tile_pool(name="const", bufs=1) as const,
            tc.tile_pool(name="xp", bufs=2) as xp,
            tc.tile_pool(name="xtp", bufs=2) as xtp,
            tc.tile_pool(name="midp", bufs=2) as midp,
            tc.tile_pool(name="outp", bufs=2) as outp,
            tc.tile_pool(name="smalls", bufs=3) as smalls,
            tc.tile_pool(name="gsm", bufs=2) as gsm,
            tc.tile_pool(name="ps_m1", bufs=2, space="PSUM") as ps_m1,
            tc.tile_pool(name="ps_m2", bufs=2, space="PSUM") as ps_m2,
            tc.tile_pool(name="ps_sm", bufs=2, space="PSUM") as ps_sm,
            tc.tile_pool(name="ps_bc", bufs=2, space="PSUM") as ps_bc,
        ):
            # ---- consts + weights on the scalar queue (its own
            # sequencer), ordered by first use; xT/x stream on sync ----
            if fp8:
                c8 = const.tile([128, 280], f8)
                nc.scalar.dma_start(out=c8, in_=cpk8.ap())
                gu8_sb = c8[:, 0:24].rearrange("p (k d) -> p k d", d=D)
                ones8_sb = c8[:, 24:280].rearrange("p (t o) -> p t o", t=2)
            w1hi_sb = const.tile([128, KCH, DF], dt1)
            nc.scalar.dma_start(out=w1hi_sb, in_=w1hi.ap())
            if fp8:
                w1lo_sb = const.tile([128, KCH, DF], f8)
                nc.scalar.dma_start(out=w1lo_sb, in_=w1lo.ap())
            r16 = const.tile([1, RO_END], f16)
            nc.scalar.dma_start(out=r16, in_=row16.ap())
            c16 = const.tile([128, 170], f16)
            nc.scalar.dma_start(out=c16, in_=cpk16.ap())
            w2gv_sb = c16[:, 0:18]
            gu16_sb = c16[:, 18:42]
            ones16_sb = c16[:, 42:170]
            cp = const.tile([128, 128], f32)
            nc.scalar.dma_start(out=cp, in_=cpack.ap())
            ident = cp[:, 0:128]
            oneh_sb = const.tile([12, 12 * 128], f16)
            nc.scalar.dma_start(out=oneh_sb, in_=oneh.ap())
            w2t_sb = const.tile([128, FCH, H], f16)
            nc.scalar.dma_start(out=w2t_sb[:, 0:3, :], in_=w2t.ap()[:, 0:3, :])
            if has_b2:
                b2bc_sb = const.tile([128, D, H], f32)
                nc.scalar.dma_start(out=b2bc_sb, in_=b2bc.ap())

            xt_first = xtp.tile([128, KCH, T], dt1, tag="xt")
            nc.sync.dma_start(out=xt_first, in_=xt_v[:, :, 0:T])

            def stage_a(mt, xt_pre=None, mid_cb=None):
                """xT + x loads, stats -> r8 (= SC/s) per sub-tile."""
                st = {}
                if xt_pre is not None:
                    xt_t = xt_pre
                else:
                    xt_t = xtp.tile([128, KCH, T], dt1, tag="xt")
                    nc.sync.dma_start(out=xt_t,
                                      in_=xt_v[:, :, mt * T:(mt + 1) * T])
                x_t = xp.tile([128, NSUB, H], f32, tag="x")
                for ss in range(NSUB):
                    nc.sync.dma_start(out=x_t[:, ss, :], in_=x_mt[mt][:, ss, :])
                    if mid_cb is not None and ss == 1:
                        mid_cb()

                # rmu[:, ss, :] = [r8=SC/s, s]
                rmu = xtp.tile([128, NSUB, 2], f32, tag="rmu")
                srow = xtp.tile([1, T], f16, tag="srow") if has_b1e else None
                for ss in range(NSUB):
                    xs = x_t[:, ss, :]
                    stt_ = smalls.tile([128, 2, 6], f32, tag="bnst")
                    nc.vector.bn_stats(out=stt_[:, 0, :], in_=xs[:, 0:512])
                    nc.vector.bn_stats(out=stt_[:, 1, :], in_=xs[:, 512:1024])
                    mv = smalls.tile([128, 2], f32, tag="mv")
                    nc.vector.bn_aggr(out=mv, in_=stt_)
                    # s = sqrt(var*H/(H-1)) + eps; r8 = SC/s
                    nc.scalar.activation(out=rmu[:, ss, 1:2], in_=mv[:, 1:2],
                                         func=AF.Sqrt, scale=float(H) / (H - 1))
                    nc.vector.tensor_scalar_add(rmu[:, ss, 1:2],
                                                rmu[:, ss, 1:2], EPS)
                    nc.vector.reciprocal(rmu[:, ss, 0:1], rmu[:, ss, 1:2])
                    if SC != 1.0:
                        nc.vector.tensor_scalar_mul(rmu[:, ss, 0:1],
                                                    rmu[:, ss, 0:1], SC)
                    if has_b1e:
                        ptr = ps_sm.tile([1, 128], f32, tag="sm")
                        nc.tensor.transpose(ptr, rmu[:, ss, 1:2], ident)
                        nc.scalar.activation(
                            out=srow[:, ss * 128:(ss + 1) * 128],
                            in_=ptr, func=AF.Copy)
                st.update(xt_t=xt_t, x_t=x_t, rmu=rmu, srow=srow)
                return st

            def b_murow(mt, st):
                """mu row [1,T] from the PE: ones^T @ xT / H."""
                xt_t = st["xt_t"]
                pmu = ps_m1.tile([128, T], f32, tag="m1")
                if fp8:
                    for k in range(KCH // 2):
                        nc.tensor.matmul(pmu, ones8_sb,
                                         xt_t[:, 2 * k:2 * k + 2, :],
                                         start=(k == 0), stop=(k == 3),
                                         perf_mode=DR)
                else:
                    for k in range(KCH):
                        nc.tensor.matmul(pmu, ones16_sb,
                                         xt_t[:, k, :],
                                         start=(k == 0), stop=(k == KCH - 1))
                murow = xtp.tile([1, T], f16, tag="murow")
                nc.scalar.activation(out=murow, in_=pmu[0:1, :], func=AF.Copy,
                                     scale=1.0 / H)
                st["murow"] = murow

            def b_chunk(mt, st, c):
                """One M1 output chunk: DR/f16 matmuls + rank-1 + relu."""
                xt_t, murow = st["xt_t"], st["murow"]
                if c == 0:
                    mid_t = midp.tile([128, FCH, T], f16, tag="mid")
                    st["mid"] = mid_t
                mid = st["mid"]
                csl = slice(c * 128, (c + 1) * 128)
                p1 = ps_m1.tile([128, T], f32, tag="m1")
                if fp8:
                    for k in range(KCH // 2):
                        nc.tensor.matmul(
                            p1, w1hi_sb[:, 2 * k:2 * k + 2, csl],
                            xt_t[:, 2 * k:2 * k + 2, :],
                            start=(k == 0), stop=False, perf_mode=DR)
                    for k in range(KCH // 2):
                        nc.tensor.matmul(
                            p1, w1lo_sb[:, 2 * k:2 * k + 2, csl],
                            xt_t[:, 2 * k:2 * k + 2, :],
                            start=False, stop=False, perf_mode=DR)
                else:
                    for k in range(KCH):
                        nc.tensor.matmul(
                            p1, w1hi_sb[:, k, csl], xt_t[:, k, :],
                            start=(k == 0), stop=False)
                w1s = r16[:, RO_W1S + c * 128:RO_W1S + (c + 1) * 128]
                nc.tensor.matmul(p1, w1s, murow,
                                 start=False, stop=not has_b1e)
                if has_b1e:
                    b1s = r16[:, RO_B1E + c * 128:RO_B1E + (c + 1) * 128]
                    nc.tensor.matmul(p1, b1s, st["srow"],
                                     start=False, stop=True)
                nc.scalar.activation(out=mid[:, c, :], in_=p1,
                                     func=AF.Relu, scale=ISC)

            def cg_ss(mt, st, ss):
                """Gate logits/sigmoid/rg for one sub-tile."""
                xt_t, mid, rmu = st["xt_t"], st["mid"], st["rmu"]
                if ss == 0:
                    rgall_t = gsm.tile([128, NSUB * D], f32, tag="rgall")
                    st["rgall"] = rgall_t
                    if has_b2:
                        gall_t = gsm.tile([128, NSUB * D], f32, tag="gall")
                        st["gall"] = gall_t
                tsl = slice(ss * 128, (ss + 1) * 128)
                pgx = ps_sm.tile([128, 3], f32, tag="sm")
                if fp8:
                    for k in range(KCH // 2):
                        nc.tensor.matmul(
                            pgx, xt_t[:, 2 * k:2 * k + 2, tsl],
                            gu8_sb[:, 2 * k:2 * k + 2, :],
                            start=(k == 0), stop=False, perf_mode=DR)
                else:
                    for k in range(KCH):
                        nc.tensor.matmul(
                            pgx, xt_t[:, k, tsl],
                            gu16_sb[:, k * D:(k + 1) * D],
                            start=(k == 0), stop=False)
                nc.tensor.matmul(pgx, r16[:, RO_ONES:RO_ONES + 128],
                                 r16[:, RO_GB:RO_GB + 3],
                                 start=False, stop=True)
                pgv = ps_sm.tile([128, 3], f32, tag="sm")
                for c in range(FCH):
                    nc.tensor.matmul(pgv, mid[:, c, tsl],
                                     w2gv_sb[:, c * D:(c + 1) * D],
                                     start=(c == 0), stop=(c == FCH - 1))
                gx = gsm.tile([128, 3], f32, tag="gx")
                nc.scalar.activation(out=gx, in_=pgx, func=AF.Copy)
                z = gsm.tile([128, 3], f32, tag="z")
                nc.vector.scalar_tensor_tensor(
                    out=z, in0=pgv, scalar=rmu[:, ss, 0:1], in1=gx,
                    op0=ALU.mult, op1=ALU.add)
                g3 = gsm.tile([128, 3], f32, tag="g3")
                nc.scalar.activation(out=g3, in_=z, func=AF.Sigmoid,
                                     scale=ISC)
                nc.gpsimd.tensor_scalar(
                    out=st["rgall"][:, ss * D:(ss + 1) * D], in0=g3,
                    scalar1=rmu[:, ss, 0:1], scalar2=ISC,
                    op0=ALU.mult, op1=ALU.mult)
                if has_b2:
                    nc.gpsimd.tensor_scalar(
                        out=st["gall"][:, ss * D:(ss + 1) * D], in0=g3,
                        scalar1=1.0, scalar2=None, op0=ALU.mult)

            def cg_fin(mt, st):
                """Transpose rg [128,12] -> [12,128] f16."""
                ptr = ps_sm.tile([NSUB * D, 128], f32, tag="sm")
                nc.tensor.transpose(ptr, st["rgall"], ident)
                rgT = gsm.tile([NSUB * D, 128], f16, tag="rgT")
                nc.scalar.activation(out=rgT, in_=ptr, func=AF.Copy)
                st["rgT"] = rgT

            def emit_bcast(mt, st, ss):
                """rg rows for sub-tile ss -> SBUF; gmid = mid*rg (Pool)."""
                mid, rgT = st["mid"], st["rgT"]
                tsl = slice(ss * 128, (ss + 1) * 128)
                gmid = midp.tile([128, FCH, 128], f16, tag=f"gmid{ss}")
                for d in range(D):
                    j = ss * D + d
                    pb = ps_bc.tile([128, 128], f32, tag="bc")
                    nc.tensor.matmul(pb, oneh_sb[:, j * 128:(j + 1) * 128],
                                     rgT, start=True, stop=True)
                    pbs = gsm.tile([128, 128], f16, tag="pbs")
                    nc.scalar.activation(out=pbs, in_=pb, func=AF.Copy)
                    for fh in range(2):
                        c = d * 2 + fh
                        nc.gpsimd.tensor_mul(gmid[:, c, :], mid[:, c, tsl],
                                             pbs)
                st[f"gmid{ss}"] = gmid

            def emit_m2(mt, st, ss):
                """M2 for sub-tile ss + residual combine + out DMA."""
                gmid, x_t = st[f"gmid{ss}"], st["x_t"]
                out_sb = outp.tile([128, H], f32, tag="osb")
                for nch in range(NCH):
                    hsl = slice(nch * 512, (nch + 1) * 512)
                    po = ps_m2.tile([128, 512], f32, tag="m2")
                    for c in range(FCH):
                        nc.tensor.matmul(po, gmid[:, c, :], w2t_sb[:, c, hsl],
                                         start=(c == 0), stop=(c == FCH - 1))
                    nc.vector.scalar_tensor_tensor(
                        out=out_sb[:, hsl], in0=x_t[:, ss, hsl],
                        scalar=2.0, in1=po, op0=ALU.mult, op1=ALU.add)
                    if has_b2:
                        for d in range(D):
                            nc.vector.scalar_tensor_tensor(
                                out=out_sb[:, hsl], in0=b2bc_sb[:, d, hsl],
                                scalar=st["gall"][:, ss * D + d:ss * D + d + 1],
                                in1=out_sb[:, hsl], op0=ALU.mult, op1=ALU.add)
                    nc.sync.dma_start(out=out_mt[mt][:, ss, hsl],
                                      in_=out_sb[:, hsl])

            def cg_interleaved(mt, st, st_next):
                """Gate chain for mt with next tile's M1 chunks as PE fill;
                the first two rg-broadcasts ride along at the end."""
                for ss in range(NSUB):
                    cg_ss(mt, st, ss)
                    b_chunk(mt + 1, st_next, ss)
                cg_fin(mt, st)
                emit_bcast(mt, st, 0)
                b_chunk(mt + 1, st_next, 4)
                emit_bcast(mt, st, 1)
                b_chunk(mt + 1, st_next, 5)

            def cbd_rest(mt, st):
                """Remaining broadcasts + all M2 (one sub-tile behind)."""
                emit_bcast(mt, st, 2)
                emit_m2(mt, st, 0)
                emit_bcast(mt, st, 3)
                emit_m2(mt, st, 1)
                emit_m2(mt, st, 2)
                emit_m2(mt, st, 3)

            # ---- software pipeline over macro-tiles ----
            def w2t_rest():
                nc.scalar.dma_start(out=w2t_sb[:, 3:6, :],
                                    in_=w2t.ap()[:, 3:6, :])

            S = [None] * NMT
            S[0] = stage_a(0, xt_pre=xt_first, mid_cb=w2t_rest)
            b_murow(0, S[0])
            for c in range(FCH):
                b_chunk(0, S[0], c)
            S[1] = stage_a(1)
            b_murow(1, S[1])
            cg_interleaved(0, S[0], S[1])
            cbd_rest(0, S[0])
            S[2] = stage_a(2)
            b_murow(2, S[2])
            cg_interleaved(1, S[1], S[2])
            cbd_rest(1, S[1])
            S[3] = stage_a(3)
            b_murow(3, S[3])
            cg_interleaved(2, S[2], S[3])
            emit_bcast(2, S[2], 2)
            emit_bcast(2, S[2], 3)
            # tail: mt3 gate chain filled with mt2's M2 work
            cg_ss(3, S[3], 0)
            emit_m2(2, S[2], 0)
            cg_ss(3, S[3], 1)
            emit_m2(2, S[2], 1)
            cg_ss(3, S[3], 2)
            emit_m2(2, S[2], 2)
            cg_ss(3, S[3], 3)
            emit_m2(2, S[2], 3)
            cg_fin(3, S[3])
            emit_bcast(3, S[3], 0)
            emit_bcast(3, S[3], 1)
            cbd_rest(3, S[3])

    _split_multiwaits(nc)
    return nc


_built = {}


def _get_nc(has_b1e, has_b2, mode=None):
    key = (has_b1e, has_b2, mode or MODE)
    if key not in _built:
        _built[key] = _build(has_b1e, has_b2, mode)
    return _built[key]


last_results = None


def kernel(x, ln_g, ln_b, W1, b1, W2, b2, gu, gv, gb):
    import ml_dtypes
    E4 = ml_dtypes.float8_e4m3

    x = np.asarray(x, dtype=np.float32)
    ln_g = np.asarray(ln_g, dtype=np.float32)
    ln_b = np.asarray(ln_b, dtype=np.float32)
    W1 = np.asarray(W1, dtype=np.float32)
    b1 = np.asarray(b1, dtype=np.float32)
    W2 = np.asarray(W2, dtype=np.float32)
    b2 = np.asarray(b2, dtype=np.float32)
    gu = np.asarray(gu, dtype=np.float32)
    gv = np.asarray(gv, dtype=np.float32)
    gb = np.asarray(gb, dtype=np.float32)

    fp8 = MODE == "fp8"
    ndt = E4 if fp8 else np.float16
    SC = 8.0 if fp8 else 1.0

    # ---- host packing (weights/layout only) ----
    W1g = np.transpose(W1, (0, 2, 1)) * ln_g[:, :, None]       # [D,H,F]
    b1e = b1 + np.einsum('dfh,dh->df', W1, ln_b)               # [D,F]
    w2gv = np.einsum('dh,dhf->df', gv, W2)                     # [D,F]
    gb_eff = gb + np.einsum('dh,dh->d', gv, b2)                # [D]
    has_b1e = bool(np.any(b1e != 0.0))
    has_b2 = bool(np.any(b2 != 0.0))

    # M1 lhsT [128, KCH, DF], chunk c=(d, fh)
    w1full = np.zeros((128, KCH, DF), np.float32)
    for c in range(FCH):
        d, fh = c // 2, c % 2
        w1full[:, :, c * 128:(c + 1) * 128] = (
            SC * W1g[d].reshape(KCH, 128, F)[:, :, fh * 128:(fh + 1) * 128]
            .transpose(1, 0, 2))
    w1hi_in = w1full.astype(ndt)
    if fp8:
        w1lo_in = (w1full - w1hi_in.astype(np.float32)).astype(E4)
    # M2 rhs [128, FCH, H]: w2t[p, c, h] = W2[d, h, fh*128+p]
    w2t_in = np.zeros((128, FCH, H), np.float16)
    for c in range(FCH):
        d, fh = c // 2, c % 2
        w2t_in[:, c, :] = W2[d, :, fh * 128:(fh + 1) * 128].T
    # block-diag w2gv [128,18] + gu chunks [128,24] + ones8 [128,8]
    c16_in = np.zeros((128, 170), np.float16)
    for c in range(FCH):
        d, fh = c // 2, c % 2
        c16_in[:, c * D + d] = w2gv[d, fh * 128:(fh + 1) * 128]
    if not fp8:
        for k in range(KCH):
            c16_in[:, 18 + k * D:18 + (k + 1) * D] = \
                (SC * gu[:, k * 128:(k + 1) * 128]).T
    c16_in[:, 42:170] = 1.0
    row16_in = np.zeros((1, RO_END), np.float16)
    row16_in[0, RO_ONES:RO_ONES + 128] = 1.0
    row16_in[0, RO_GB:RO_GB + D] = SC * gb_eff
    row16_in[0, RO_W1S:RO_W1S + DF] = -w1full.sum(axis=(0, 1))
    if has_b1e:
        b1e_pack = np.zeros(DF, np.float32)
        for c in range(FCH):
            d, fh = c // 2, c % 2
            b1e_pack[c * 128:(c + 1) * 128] = \
                SC * b1e[d, fh * 128:(fh + 1) * 128]
        row16_in[0, RO_B1E:RO_B1E + DF] = b1e_pack
    oneh_in = np.zeros((12, 12 * 128), np.float16)
    for j in range(12):
        oneh_in[j, j * 128:(j + 1) * 128] = 1.0
    cpack_in = np.eye(128, dtype=np.float32)

    common = {
        "w1hi": w1hi_in, "w2t": w2t_in, "cpack": cpack_in,
        "cpk16": c16_in, "row16": row16_in, "oneh": oneh_in,
    }
    if fp8:
        common["w1lo"] = w1lo_in
        cpk8_in = np.zeros((128, 280), np.float32)
        for k in range(KCH):
            cpk8_in[:, k * D:(k + 1) * D] = \
                (SC * gu[:, k * 128:(k + 1) * 128]).T
        cpk8_in[:, 24:280] = 1.0
        common["cpk8"] = cpk8_in.astype(E4)
    if has_b2:
        common["b2bc"] = np.broadcast_to(
            b2[None, :, :], (128, D, H)).astype(np.float32).copy()

    nc = _get_nc(has_b1e, has_b2)

    in_maps = []
    for c in range(B):
        m = dict(common, xin=np.ascontiguousarray(x[c]))
        m["xt"] = np.ascontiguousarray(x[c].T).astype(ndt)
        in_maps.append(m)
    res = run_bass_kernel_spmd(nc, in_maps, core_ids=list(range(B)))
    global last_results
    last_results = res
    return np.stack([res.results[c]["out"] for c in range(B)])


# revision 11
# speedup vs baseline: 1.0333x; 1.0333x over previous
"""Trainium2 Bass kernel for nn_MixtureOfAdapter (moe_routing), v3.

Math (per token, H=1024, F=256, D=3 domains; grading inputs have
ln_g=1, ln_b=0, b1=0, b2=0, gb=0):
    mu, sd (ddof=1) over H;  s = sd + eps;  xn = (x - mu)/s
    mid_d = relu(W1g_d xn + b1e_d);  a_d = W2_d mid_d + b2_d
    gate_d = sigmoid(gu_d.x + gv_d.a_d + gb_d)
    out = 2x + sum_d gate_d * a_d

Kernel strategy (8 cores, data-parallel over batch B=8):
  - Ship TWO copies of x per core: natural f32 [L,H] (stats via
    bn_stats + final residual) and a host-transposed [H,L] copy in a
    compact dtype (f16, or fp8e4 in fp8 mode) that feeds all matmuls.
    No PE transposes of x, no on-device centering.
  - M1 runs on UNCENTERED xT; centering folds into a rank-1
    correction: out1 = W1q^T xq - colsum(W1q)*mu.  mu itself comes
    from the PE (ones^T @ xT / H), so M1 never waits on the natural-x
    DMA or the stats chain.  relu(out1) = s*mid ("mid_s"), f16.
  - fp8 mode: W1 split hi+lo fp8e4 (noise-free weights), xT single
    fp8; DoubleRow matmuls (0.5 cycles/row, 2 k-chunks/pass) = 4x
    f32r rate.  SC=8 on (W1, gu, gb); relu/sigmoid absorb 1/SC.
  - Gates in [token, domain] layout: pgux[t,d] + gb rank-1; pgv[t,d]
    from mid chunks; z = pgv*r8 + pgux (DVE); gate = sigmoid(z/SC);
    rg = gate*r -> [128,12] -> one PE transpose -> one-hot broadcast
    per (ss,d) -> Act copy to SBUF -> gmid = mid*rg on Pool (f16).
  - M2 (f16) accumulates all domains into one PSUM per (ss, half);
    out = 2x + pout via one DVE scalar_tensor_tensor per half.
  - Emission interleaves next tile's M1 chunks into the gate chain so
    the PE FIFO never drains behind DVE/Act/Pool latency.
"""

import numpy as np

import concourse.bass as bass
import concourse.mybir as mybir
import concourse.tile as tile
from concourse.bass_utils import run_bass_kernel_spmd

B, L, H, F, D = 8, 2048, 1024, 256, 3
EPS = 1e-6
T = 512                 # tokens per macro-tile
NSUB = T // 128         # 4 sub-tiles of 128 tokens
NMT = L // T            # 4 macro-tiles per core
KCH = H // 128          # 8 k-chunks over H
FCH = (D * F) // 128    # 6 chunks over stacked (domain, F)
NCH = H // 512          # 2 output column chunks
DF = D * F

f32 = mybir.dt.float32
f16 = mybir.dt.float16
f8 = mybir.dt.float8e4
AF = mybir.ActivationFunctionType
ALU = mybir.AluOpType
DR = mybir.MatmulPerfMode.DoubleRow

MODE = "fp8"            # "fp8" (DoubleRow M1) or "f16"

# row16 layout (single-partition f16 consts)
RO_ONES = 0             # [0:128] ones
RO_GB = 128             # [128:131] SC*gb_eff
RO_W1S = 131            # [131:899] -SC*colsum(W1g)
RO_B1E = 899            # [899:1667] SC*b1e
RO_END = 1667


def _split_multiwaits(nc):
    """This walrus build allows 1 sync-wait per instruction (2 for
    EventSemaphore); Tile can attach more.  Move extras onto preceding
    same-engine NoOps (engine queues are FIFO, so semantics identical)."""
    for fn in nc.m.functions:
        for bb in fn.blocks:
            new = []
            changed = False
            for inst in bb.instructions:
                si = inst.sync_info
                cap = 2 if isinstance(inst, mybir.InstEventSemaphore) else 1
                if si is not None and len(si.on_wait) > cap:
                    waits = list(si.on_wait)
                    extra, kept = waits[:-cap], waits[-cap:]
                    for j, w in enumerate(extra):
                        new.append(mybir.InstNoOp(
                            name=f"{inst.name}-wsplit{j}",
                            engine=inst.engine,
                            sync_info=mybir.SyncInfo(on_wait=[w], on_update=[]),
                            ins=[], outs=[],
                        ))
                    inst.sync_info = mybir.SyncInfo(
                        on_wait=kept, on_update=list(si.on_update))
                    changed = True
                new.append(inst)
            if changed:
                bb.instructions = new


def _build(has_b1e: bool, has_b2: bool, mode: str = None):
    mode = mode or MODE
    fp8 = mode == "fp8"
    dt1 = f8 if fp8 else f16
    SC = 8.0 if fp8 else 1.0
    ISC = 1.0 / SC

    nc = bass.Bass(target_bir_lowering=False)

    xin = nc.dram_tensor("xin", [L, H], f32, kind="ExternalInput")
    xt = nc.dram_tensor("xt", [H, L], dt1, kind="ExternalInput")
    w1hi = nc.dram_tensor("w1hi", [128, KCH, DF], dt1, kind="ExternalInput")
    if fp8:
        w1lo = nc.dram_tensor("w1lo", [128, KCH, DF], f8, kind="ExternalInput")
    w2t = nc.dram_tensor("w2t", [128, FCH, H], f16, kind="ExternalInput")
    cpack = nc.dram_tensor("cpack", [128, 128], f32, kind="ExternalInput")
    cpk16 = nc.dram_tensor("cpk16", [128, 170], f16, kind="ExternalInput")
    row16 = nc.dram_tensor("row16", [1, RO_END], f16, kind="ExternalInput")
    oneh = nc.dram_tensor("oneh", [12, 12 * 128], f16, kind="ExternalInput")
    if fp8:
        cpk8 = nc.dram_tensor("cpk8", [128, 280], f8, kind="ExternalInput")
    if has_b2:
        b2bc = nc.dram_tensor("b2bc", [128, D, H], f32, kind="ExternalInput")
    out = nc.dram_tensor("out", [L, H], f32, kind="ExternalOutput")

    x_mt = xin.ap().rearrange("(m s p) h -> m p s h", p=128, s=NSUB)
    xt_v = xt.ap().rearrange("(k p) l -> p k l", p=128)
    out_mt = out.ap().rearrange("(m s p) h -> m p s h", p=128, s=NSUB)

    with tile.TileContext(nc) as tc:
        with (
            tc.tile_pool(name="const", bufs=1) as const,
            tc.tile_pool(name="xp", bufs=2) as xp,
            tc.tile_pool(name="xtp", bufs=2) as xtp,
            tc.tile_pool(name="midp", bufs=2) as midp,
            tc.tile_pool(name="outp", bufs=2) as outp,
            tc.tile_pool(name="smalls", bufs=3) as smalls,
            tc.tile_pool(name="gsm", bufs=2) as gsm,
            tc.tile_pool(name="ps_m1", bufs=2, space="PSUM") as ps_m1,
            tc.tile_pool(name="ps_m2", bufs=2, space="PSUM") as ps_m2,
            tc.tile_pool(name="ps_sm", bufs=2, space="PSUM") as ps_sm,
            tc.tile_pool(name="ps_bc", bufs=2, space="PSUM") as ps_bc,
        ):
            # ---- consts + weights on the scalar queue (its own
            # sequencer), ordered by first use; xT/x stream on sync ----
            if fp8:
                c8 = const.tile([128, 280], f8)
                nc.gpsimd.dma_start(out=c8, in_=cpk8.ap())
                gu8_sb = c8[:, 0:24].rearrange("p (k d) -> p k d", d=D)
                ones8_sb = c8[:, 24:280].rearrange("p (t o) -> p t o", t=2)
            w1hi_sb = const.tile([128, KCH, DF], dt1)
            nc.gpsimd.dma_start(out=w1hi_sb, in_=w1hi.ap())
            if fp8:
                w1lo_sb = const.tile([128, KCH, DF], f8)
                nc.gpsimd.dma_start(out=w1lo_sb, in_=w1lo.ap())
            r16 = const.tile([1, RO_END], f16)
            nc.gpsimd.dma_start(out=r16, in_=row16.ap())
            c16 = const.tile([128, 170], f16)
            nc.scalar.dma_start(out=c16, in_=cpk16.ap())
            w2gv_sb = c16[:, 0:18]
            gu16_sb = c16[:, 18:42]
            ones16_sb = c16[:, 42:170]
            cp = const.tile([128, 128], f32)
            nc.scalar.dma_start(out=cp, in_=cpack.ap())
            ident = cp[:, 0:128]
            oneh_sb = const.tile([12, 12 * 128], f16)
            nc.scalar.dma_start(out=oneh_sb, in_=oneh.ap())
            w2t_sb = const.tile([128, FCH, H], f16)
            nc.scalar.dma_start(out=w2t_sb[:, 0:3, :], in_=w2t.ap()[:, 0:3, :])
            if has_b2:
                b2bc_sb = const.tile([128, D, H], f32)
                nc.scalar.dma_start(out=b2bc_sb, in_=b2bc.ap())

            xt_first = xtp.tile([128, KCH, T], dt1, tag="xt")
            nc.sync.dma_start(out=xt_first, in_=xt_v[:, :, 0:T])

            def stage_a(mt, xt_pre=None, mid_cb=None):
                """xT + x loads, stats -> r8 (= SC/s) per sub-tile."""
                st = {}
                if xt_pre is not None:
                    xt_t = xt_pre
                else:
                    xt_t = xtp.tile([128, KCH, T], dt1, tag="xt")
                    nc.sync.dma_start(out=xt_t,
                                      in_=xt_v[:, :, mt * T:(mt + 1) * T])
                x_t = xp.tile([128, NSUB, H], f32, tag="x")
                for ss in range(NSUB):
                    nc.sync.dma_start(out=x_t[:, ss, :], in_=x_mt[mt][:, ss, :])
                    if mid_cb is not None and ss == 1:
                        mid_cb()

                # rmu[:, ss, :] = [r8=SC/s, s]
                rmu = xtp.tile([128, NSUB, 2], f32, tag="rmu")
                srow = xtp.tile([1, T], f16, tag="srow") if has_b1e else None
                for ss in range(NSUB):
                    xs = x_t[:, ss, :]
                    stt_ = smalls.tile([128, 2, 6], f32, tag="bnst")
                    nc.vector.bn_stats(out=stt_[:, 0, :], in_=xs[:, 0:512])
                    nc.vector.bn_stats(out=stt_[:, 1, :], in_=xs[:, 512:1024])
                    mv = smalls.tile([128, 2], f32, tag="mv")
                    nc.vector.bn_aggr(out=mv, in_=stt_)
                    # s = sqrt(var*H/(H-1)) + eps; r8 = SC/s
                    nc.scalar.activation(out=rmu[:, ss, 1:2], in_=mv[:, 1:2],
                                         func=AF.Sqrt, scale=float(H) / (H - 1))
                    nc.vector.tensor_scalar_add(rmu[:, ss, 1:2],
                                                rmu[:, ss, 1:2], EPS)
                    nc.vector.reciprocal(rmu[:, ss, 0:1], rmu[:, ss, 1:2])
                    if SC != 1.0:
                        nc.vector.tensor_scalar_mul(rmu[:, ss, 0:1],
                                                    rmu[:, ss, 0:1], SC)
                    if has_b1e:
                        ptr = ps_sm.tile([1, 128], f32, tag="sm")
                        nc.tensor.transpose(ptr, rmu[:, ss, 1:2], ident)
                        nc.scalar.activation(
                            out=srow[:, ss * 128:(ss + 1) * 128],
                            in_=ptr, func=AF.Copy)
                st.update(xt_t=xt_t, x_t=x_t, rmu=rmu, srow=srow)
                return st

            def b_murow(mt, st):
                """mu row [1,T] from the PE: ones^T @ xT / H."""
                xt_t = st["xt_t"]
                pmu = ps_m1.tile([128, T], f32, tag="m1")
                if fp8:
                    for k in range(KCH // 2):
                        nc.tensor.matmul(pmu, ones8_sb,
                                         xt_t[:, 2 * k:2 * k + 2, :],
                                         start=(k == 0), stop=(k == 3),
                                         perf_mode=DR)
                else:
                    for k in range(KCH):
                        nc.tensor.matmul(pmu, ones16_sb,
                                         xt_t[:, k, :],
                                         start=(k == 0), stop=(k == KCH - 1))
                murow = xtp.tile([1, T], f16, tag="murow")
                nc.scalar.activation(out=murow, in_=pmu[0:1, :], func=AF.Copy,
                                     scale=1.0 / H)
                st["murow"] = murow

            def b_chunk(mt, st, c):
                """One M1 output chunk: DR/f16 matmuls + rank-1 + relu."""
                xt_t, murow = st["xt_t"], st["murow"]
                if c == 0:
                    mid_t = midp.tile([128, FCH, T], f16, tag="mid")
                    st["mid"] = mid_t
                mid = st["mid"]
                csl = slice(c * 128, (c + 1) * 128)
                p1 = ps_m1.tile([128, T], f32, tag="m1")
                if fp8:
                    for k in range(KCH // 2):
                        nc.tensor.matmul(
                            p1, w1hi_sb[:, 2 * k:2 * k + 2, csl],
                            xt_t[:, 2 * k:2 * k + 2, :],
                            start=(k == 0), stop=False, perf_mode=DR)
                    for k in range(KCH // 2):
                        nc.tensor.matmul(
                            p1, w1lo_sb[:, 2 * k:2 * k + 2, csl],
                            xt_t[:, 2 * k:2 * k + 2, :],
                            start=False, stop=False, perf_mode=DR)
                else:
                    for k in range(KCH):
                        nc.tensor.matmul(
                            p1, w1hi_sb[:, k, csl], xt_t[:, k, :],
                            start=(k == 0), stop=False)
                w1s = r16[:, RO_W1S + c * 128:RO_W1S + (c + 1) * 128]
                nc.tensor.matmul(p1, w1s, murow,
                                 start=False, stop=not has_b1e)
                if has_b1e:
                    b1s = r16[:, RO_B1E + c * 128:RO_B1E + (c + 1) * 128]
                    nc.tensor.matmul(p1, b1s, st["srow"],
                                     start=False, stop=True)
                nc.scalar.activation(out=mid[:, c, :], in_=p1,
                                     func=AF.Relu, scale=ISC)

            def cg_ss(mt, st, ss):
                """Gate logits/sigmoid/rg for one sub-tile."""
                xt_t, mid, rmu = st["xt_t"], st["mid"], st["rmu"]
                if ss == 0:
                    rgall_t = gsm.tile([128, NSUB * D], f32, tag="rgall")
                    st["rgall"] = rgall_t
                    if has_b2:
                        gall_t = gsm.tile([128, NSUB * D], f32, tag="gall")
                        st["gall"] = gall_t
                tsl = slice(ss * 128, (ss + 1) * 128)
                pgx = ps_sm.tile([128, 3], f32, tag="sm")
                if fp8:
                    for k in range(KCH // 2):
                        nc.tensor.matmul(
                            pgx, xt_t[:, 2 * k:2 * k + 2, tsl],
                            gu8_sb[:, 2 * k:2 * k + 2, :],
                            start=(k == 0), stop=False, perf_mode=DR)
                else:
                    for k in range(KCH):
                        nc.tensor.matmul(
                            pgx, xt_t[:, k, tsl],
                            gu16_sb[:, k * D:(k + 1) * D],
                            start=(k == 0), stop=False)
                nc.tensor.matmul(pgx, r16[:, RO_ONES:RO_ONES + 128],
                                 r16[:, RO_GB:RO_GB + 3],
                                 start=False, stop=True)
                pgv = ps_sm.tile([128, 3], f32, tag="sm")
                for c in range(FCH):
                    nc.tensor.matmul(pgv, mid[:, c, tsl],
                                     w2gv_sb[:, c * D:(c + 1) * D],
                                     start=(c == 0), stop=(c == FCH - 1))
                gx = gsm.tile([128, 3], f32, tag="gx")
                nc.scalar.activation(out=gx, in_=pgx, func=AF.Copy)
                z = gsm.tile([128, 3], f32, tag="z")
                nc.vector.scalar_tensor_tensor(
                    out=z, in0=pgv, scalar=rmu[:, ss, 0:1], in1=gx,
                    op0=ALU.mult, op1=ALU.add)
                g3 = gsm.tile([128, 3], f32, tag="g3")
                nc.scalar.activation(out=g3, in_=z, func=AF.Sigmoid,
                                     scale=ISC)
                nc.gpsimd.tensor_scalar(
                    out=st["rgall"][:, ss * D:(ss + 1) * D], in0=g3,
                    scalar1=rmu[:, ss, 0:1], scalar2=ISC,
                    op0=ALU.mult, op1=ALU.mult)
                if has_b2:
                    nc.gpsimd.tensor_scalar(
                        out=st["gall"][:, ss * D:(ss + 1) * D], in0=g3,
                        scalar1=1.0, scalar2=None, op0=ALU.mult)

            def cg_fin(mt, st):
                """Transpose rg [128,12] -> [12,128] f16."""
                ptr = ps_sm.tile([NSUB * D, 128], f32, tag="sm")
                nc.tensor.transpose(ptr, st["rgall"], ident)
                rgT = gsm.tile([NSUB * D, 128], f16, tag="rgT")
                nc.scalar.activation(out=rgT, in_=ptr, func=AF.Copy)
                st["rgT"] = rgT

            def emit_bcast(mt, st, ss):
                """rg rows for sub-tile ss -> SBUF; gmid = mid*rg (Pool)."""
                mid, rgT = st["mid"], st["rgT"]
                tsl = slice(ss * 128, (ss + 1) * 128)
                gmid = midp.tile([128, FCH, 128], f16, tag=f"gmid{ss}")
                for d in range(D):
                    j = ss * D + d
                    pb = ps_bc.tile([128, 128], f32, tag="bc")
                    nc.tensor.matmul(pb, oneh_sb[:, j * 128:(j + 1) * 128],
                                     rgT, start=True, stop=True)
                    pbs = gsm.tile([128, 128], f16, tag="pbs")
                    nc.scalar.activation(out=pbs, in_=pb, func=AF.Copy)
                    for fh in range(2):
                        c = d * 2 + fh
                        nc.gpsimd.tensor_mul(gmid[:, c, :], mid[:, c, tsl],
                                             pbs)
                st[f"gmid{ss}"] = gmid

            def emit_m2(mt, st, ss):
                """M2 for sub-tile ss + residual combine + out DMA."""
                gmid, x_t = st[f"gmid{ss}"], st["x_t"]
                out_sb = outp.tile([128, H], f32, tag="osb")
                for nch in range(NCH):
                    hsl = slice(nch * 512, (nch + 1) * 512)
                    po = ps_m2.tile([128, 512], f32, tag="m2")
                    for c in range(FCH):
                        nc.tensor.matmul(po, gmid[:, c, :], w2t_sb[:, c, hsl],
                                         start=(c == 0), stop=(c == FCH - 1))
                    nc.vector.scalar_tensor_tensor(
                        out=out_sb[:, hsl], in0=x_t[:, ss, hsl],
                        scalar=2.0, in1=po, op0=ALU.mult, op1=ALU.add)
                    if has_b2:
                        for d in range(D):
                            nc.vector.scalar_tensor_tensor(
                                out=out_sb[:, hsl], in0=b2bc_sb[:, d, hsl],
                                scalar=st["gall"][:, ss * D + d:ss * D + d + 1],
                                in1=out_sb[:, hsl], op0=ALU.mult, op1=ALU.add)
                    nc.sync.dma_start(out=out_mt[mt][:, ss, hsl],
                                      in_=out_sb[:, hsl])

            def cg_interleaved(mt, st, st_next):
                """Gate chain for mt with next tile's M1 chunks as PE fill;
                the first two rg-broadcasts ride along at the end."""
                for ss in range(NSUB):
                    cg_ss(mt, st, ss)
                    b_chunk(mt + 1, st_next, ss)
                cg_fin(mt, st)
                emit_bcast(mt, st, 0)
                b_chunk(mt + 1, st_next, 4)
                emit_bcast(mt, st, 1)
                b_chunk(mt + 1, st_next, 5)

            def cbd_rest(mt, st):
                """Remaining broadcasts + all M2 (one sub-tile behind)."""
                emit_bcast(mt, st, 2)
                emit_m2(mt, st, 0)
                emit_bcast(mt, st, 3)
                emit_m2(mt, st, 1)
                emit_m2(mt, st, 2)
                emit_m2(mt, st, 3)

            # ---- software pipeline over macro-tiles ----
            def w2t_rest():
                nc.scalar.dma_start(out=w2t_sb[:, 3:6, :],
                                    in_=w2t.ap()[:, 3:6, :])

            S = [None] * NMT
            S[0] = stage_a(0, xt_pre=xt_first, mid_cb=w2t_rest)
            b_murow(0, S[0])
            for c in range(FCH):
                b_chunk(0, S[0], c)
            S[1] = stage_a(1)
            b_murow(1, S[1])
            cg_interleaved(0, S[0], S[1])
            cbd_rest(0, S[0])
            S[2] = stage_a(2)
            b_murow(2, S[2])
            cg_interleaved(1, S[1], S[2])
            cbd_rest(1, S[1])
            S[3] = stage_a(3)
            b_murow(3, S[3])
            cg_interleaved(2, S[2], S[3])
            emit_bcast(2, S[2], 2)
            emit_bcast(2, S[2], 3)
            # tail: mt3 gate chain filled with mt2's M2 work
            cg_ss(3, S[3], 0)
            emit_m2(2, S[2], 0)
            cg_ss(3, S[3], 1)
            emit_m2(2, S[2], 1)
            cg_ss(3, S[3], 2)
            emit_m2(2, S[2], 2)
            cg_ss(3, S[3], 3)
            emit_m2(2, S[2], 3)
            cg_fin(3, S[3])
            emit_bcast(3, S[3], 0)
            emit_bcast(3, S[3], 1)
            cbd_rest(3, S[3])

    _split_multiwaits(nc)
    return nc


_built = {}


def _get_nc(has_b1e, has_b2, mode=None):
    key = (has_b1e, has_b2, mode or MODE)
    if key not in _built:
        _built[key] = _build(has_b1e, has_b2, mode)
    return _built[key]


last_results = None


def kernel(x, ln_g, ln_b, W1, b1, W2, b2, gu, gv, gb):
    import ml_dtypes
    E4 = ml_dtypes.float8_e4m3

    x = np.asarray(x, dtype=np.float32)
    ln_g = np.asarray(ln_g, dtype=np.float32)
    ln_b = np.asarray(ln_b, dtype=np.float32)
    W1 = np.asarray(W1, dtype=np.float32)
    b1 = np.asarray(b1, dtype=np.float32)
    W2 = np.asarray(W2, dtype=np.float32)
    b2 = np.asarray(b2, dtype=np.float32)
    gu = np.asarray(gu, dtype=np.float32)
    gv = np.asarray(gv, dtype=np.float32)
    gb = np.asarray(gb, dtype=np.float32)

    fp8 = MODE == "fp8"
    ndt = E4 if fp8 else np.float16
    SC = 8.0 if fp8 else 1.0

    # ---- host packing (weights/layout only) ----
    W1g = np.transpose(W1, (0, 2, 1)) * ln_g[:, :, None]       # [D,H,F]
    b1e = b1 + np.einsum('dfh,dh->df', W1, ln_b)               # [D,F]
    w2gv = np.einsum('dh,dhf->df', gv, W2)                     # [D,F]
    gb_eff = gb + np.einsum('dh,dh->d', gv, b2)                # [D]
    has_b1e = bool(np.any(b1e != 0.0))
    has_b2 = bool(np.any(b2 != 0.0))

    # M1 lhsT [128, KCH, DF], chunk c=(d, fh)
    w1full = np.zeros((128, KCH, DF), np.float32)
    for c in range(FCH):
        d, fh = c // 2, c % 2
        w1full[:, :, c * 128:(c + 1) * 128] = (
            SC * W1g[d].reshape(KCH, 128, F)[:, :, fh * 128:(fh + 1) * 128]
            .transpose(1, 0, 2))
    w1hi_in = w1full.astype(ndt)
    if fp8:
        w1lo_in = (w1full - w1hi_in.astype(np.float32)).astype(E4)
    # M2 rhs [128, FCH, H]: w2t[p, c, h] = W2[d, h, fh*128+p]
    w2t_in = np.zeros((128, FCH, H), np.float16)
    for c in range(FCH):
        d, fh = c // 2, c % 2
        w2t_in[:, c, :] = W2[d, :, fh * 128:(fh + 1) * 128].T
    # block-diag w2gv [128,18] + gu chunks [128,24] + ones8 [128,8]
    c16_in = np.zeros((128, 170), np.float16)
    for c in range(FCH):
        d, fh = c // 2, c % 2
        c16_in[:, c * D + d] = w2gv[d, fh * 128:(fh + 1) * 128]
    if not fp8:
        for k in range(KCH):
            c16_in[:, 18 + k * D:18 + (k + 1) * D] = \
                (SC * gu[:, k * 128:(k + 1) * 128]).T
    c16_in[:, 42:170] = 1.0
    row16_in = np.zeros((1, RO_END), np.float16)
    row16_in[0, RO_ONES:RO_ONES + 128] = 1.0
    row16_in[0, RO_GB:RO_GB + D] = SC * gb_eff
    row16_in[0, RO_W1S:RO_W1S + DF] = -w1full.sum(axis=(0, 1))
    if has_b1e:
        b1e_pack = np.zeros(DF, np.float32)
        for c in range(FCH):
            d, fh = c // 2, c % 2
            b1e_pack[c * 128:(c + 1) * 128] = \
                SC * b1e[d, fh * 128:(fh + 1) * 128]
        row16_in[0, RO_B1E:RO_B1E + DF] = b1e_pack
    oneh_in = np.zeros((12, 12 * 128), np.float16)
    for j in range(12):
        oneh_in[j, j * 128:(j + 1) * 128] = 1.0
    cpack_in = np.eye(128, dtype=np.float32)

    common = {
        "w1hi": w1hi_in, "w2t": w2t_in, "cpack": cpack_in,
        "cpk16": c16_in, "row16": row16_in, "oneh": oneh_in,
    }
    if fp8:
        common["w1lo"] = w1lo_in
        cpk8_in = np.zeros((128, 280), np.float32)
        for k in range(KCH):
            cpk8_in[:, k * D:(k + 1) * D] = \
                (SC * gu[:, k * 128:(k + 1) * 128]).T
        cpk8_in[:, 24:280] = 1.0
        common["cpk8"] = cpk8_in.astype(E4)
    if has_b2:
        common["b2bc"] = np.broadcast_to(
            b2[None, :, :], (128, D, H)).astype(np.float32).copy()

    nc = _get_nc(has_b1e, has_b2)

    in_maps = []
    for c in range(B):
        m = dict(common, xin=np.ascontiguousarray(x[c]))
        m["xt"] = np.ascontiguousarray(x[c].T).astype(ndt)
        in_maps.append(m)
    res = run_bass_kernel_spmd(nc, in_maps, core_ids=list(range(B)))
    global last_results
    last_results = res
    return np.stack([res.results[c]["out"] for c in range(B)])


# revision 12
# speedup vs baseline: 1.0661x; 1.0318x over previous
"""Trainium2 Bass kernel for nn_MixtureOfAdapter (moe_routing), v3.

Math (per token, H=1024, F=256, D=3 domains; grading inputs have
ln_g=1, ln_b=0, b1=0, b2=0, gb=0):
    mu, sd (ddof=1) over H;  s = sd + eps;  xn = (x - mu)/s
    mid_d = relu(W1g_d xn + b1e_d);  a_d = W2_d mid_d + b2_d
    gate_d = sigmoid(gu_d.x + gv_d.a_d + gb_d)
    out = 2x + sum_d gate_d * a_d

Kernel strategy (8 cores, data-parallel over batch B=8):
  - Ship TWO copies of x per core: natural f32 [L,H] (stats via
    bn_stats + final residual) and a host-transposed [H,L] copy in a
    compact dtype (f16, or fp8e4 in fp8 mode) that feeds all matmuls.
    No PE transposes of x, no on-device centering.
  - M1 runs on UNCENTERED xT; centering folds into a rank-1
    correction: out1 = W1q^T xq - colsum(W1q)*mu.  mu itself comes
    from the PE (ones^T @ xT / H), so M1 never waits on the natural-x
    DMA or the stats chain.  relu(out1) = s*mid ("mid_s"), f16.
  - fp8 mode: W1 split hi+lo fp8e4 (noise-free weights), xT single
    fp8; DoubleRow matmuls (0.5 cycles/row, 2 k-chunks/pass) = 4x
    f32r rate.  SC=8 on (W1, gu, gb); relu/sigmoid absorb 1/SC.
  - Gates in [token, domain] layout: pgux[t,d] + gb rank-1; pgv[t,d]
    from mid chunks; z = pgv*r8 + pgux (DVE); gate = sigmoid(z/SC);
    rg = gate*r -> [128,12] -> one PE transpose -> one-hot broadcast
    per (ss,d) -> Act copy to SBUF -> gmid = mid*rg on Pool (f16).
  - M2 (f16) accumulates all domains into one PSUM per (ss, half);
    out = 2x + pout via one DVE scalar_tensor_tensor per half.
  - Emission interleaves next tile's M1 chunks into the gate chain so
    the PE FIFO never drains behind DVE/Act/Pool latency.
"""

import numpy as np

import concourse.bass as bass
import concourse.mybir as mybir
import concourse.tile as tile
from concourse.bass_utils import run_bass_kernel_spmd

B, L, H, F, D = 8, 2048, 1024, 256, 3
EPS = 1e-6
T = 512                 # tokens per macro-tile
NSUB = T // 128         # 4 sub-tiles of 128 tokens
NMT = L // T            # 4 macro-tiles per core
KCH = H // 128          # 8 k-chunks over H
FCH = (D * F) // 128    # 6 chunks over stacked (domain, F)
NCH = H // 512          # 2 output column chunks
DF = D * F

f32 = mybir.dt.float32
f16 = mybir.dt.float16
f8 = mybir.dt.float8e4
AF = mybir.ActivationFunctionType
ALU = mybir.AluOpType
DR = mybir.MatmulPerfMode.DoubleRow

MODE = "fp8"            # "fp8" (DoubleRow M1) or "f16"

# row16 layout (single-partition f16 consts)
RO_ONES = 0             # [0:128] ones
RO_GB = 128             # [128:131] SC*gb_eff
RO_W1S = 131            # [131:899] -SC*colsum(W1g)
RO_B1E = 899            # [899:1667] SC*b1e
RO_END = 1667


def _split_multiwaits(nc):
    """This walrus build allows 1 sync-wait per instruction (2 for
    EventSemaphore); Tile can attach more.  Move extras onto preceding
    same-engine NoOps (engine queues are FIFO, so semantics identical)."""
    for fn in nc.m.functions:
        for bb in fn.blocks:
            new = []
            changed = False
            for inst in bb.instructions:
                si = inst.sync_info
                cap = 2 if isinstance(inst, mybir.InstEventSemaphore) else 1
                if si is not None and len(si.on_wait) > cap:
                    waits = list(si.on_wait)
                    extra, kept = waits[:-cap], waits[-cap:]
                    for j, w in enumerate(extra):
                        new.append(mybir.InstNoOp(
                            name=f"{inst.name}-wsplit{j}",
                            engine=inst.engine,
                            sync_info=mybir.SyncInfo(on_wait=[w], on_update=[]),
                            ins=[], outs=[],
                        ))
                    inst.sync_info = mybir.SyncInfo(
                        on_wait=kept, on_update=list(si.on_update))
                    changed = True
                new.append(inst)
            if changed:
                bb.instructions = new


def _build(has_b1e: bool, has_b2: bool, mode: str = None):
    mode = mode or MODE
    fp8 = mode == "fp8"
    dt1 = f8 if fp8 else f16
    SC = 8.0 if fp8 else 1.0
    ISC = 1.0 / SC

    nc = bass.Bass(target_bir_lowering=False)

    xin = nc.dram_tensor("xin", [L, H], f32, kind="ExternalInput")
    xt = nc.dram_tensor("xt", [H, L], dt1, kind="ExternalInput")
    w1hi = nc.dram_tensor("w1hi", [128, KCH, DF], dt1, kind="ExternalInput")
    if fp8:
        w1lo = nc.dram_tensor("w1lo", [128, KCH, DF], f8, kind="ExternalInput")
    w2t = nc.dram_tensor("w2t", [128, FCH, H], f16, kind="ExternalInput")
    cpack = nc.dram_tensor("cpack", [128, 128], f32, kind="ExternalInput")
    cpk16 = nc.dram_tensor("cpk16", [128, 170], f16, kind="ExternalInput")
    row16 = nc.dram_tensor("row16", [1, RO_END], f16, kind="ExternalInput")
    oneh = nc.dram_tensor("oneh", [12, 12 * 128], f16, kind="ExternalInput")
    if fp8:
        cpk8 = nc.dram_tensor("cpk8", [128, 280], f8, kind="ExternalInput")
    if has_b2:
        b2bc = nc.dram_tensor("b2bc", [128, D, H], f32, kind="ExternalInput")
    out = nc.dram_tensor("out", [L, H], f32, kind="ExternalOutput")

    x_mt = xin.ap().rearrange("(m s p) h -> m p s h", p=128, s=NSUB)
    xt_v = xt.ap().rearrange("(k p) l -> p k l", p=128)
    out_mt = out.ap().rearrange("(m s p) h -> m p s h", p=128, s=NSUB)

    with tile.TileContext(nc) as tc:
        with (
            tc.tile_pool(name="const", bufs=1) as const,
            tc.tile_pool(name="xp", bufs=2) as xp,
            tc.tile_pool(name="xtp", bufs=2) as xtp,
            tc.tile_pool(name="midp", bufs=2) as midp,
            tc.tile_pool(name="outp", bufs=2) as outp,
            tc.tile_pool(name="smalls", bufs=3) as smalls,
            tc.tile_pool(name="gsm", bufs=2) as gsm,
            tc.tile_pool(name="ps_m1", bufs=2, space="PSUM") as ps_m1,
            tc.tile_pool(name="ps_m2", bufs=2, space="PSUM") as ps_m2,
            tc.tile_pool(name="ps_sm", bufs=2, space="PSUM") as ps_sm,
            tc.tile_pool(name="ps_bc", bufs=2, space="PSUM") as ps_bc,
        ):
            # ---- consts + weights on the scalar queue (its own
            # sequencer), ordered by first use; xT/x stream on sync ----
            if fp8:
                c8 = const.tile([128, 280], f8)
                nc.gpsimd.dma_start(out=c8, in_=cpk8.ap())
                gu8_sb = c8[:, 0:24].rearrange("p (k d) -> p k d", d=D)
                ones8_sb = c8[:, 24:280].rearrange("p (t o) -> p t o", t=2)
            w1hi_sb = const.tile([128, KCH, DF], dt1)
            nc.gpsimd.dma_start(out=w1hi_sb, in_=w1hi.ap())
            if fp8:
                w1lo_sb = const.tile([128, KCH, DF], f8)
                nc.gpsimd.dma_start(out=w1lo_sb, in_=w1lo.ap())
            r16 = const.tile([1, RO_END], f16)
            nc.gpsimd.dma_start(out=r16, in_=row16.ap())
            c16 = const.tile([128, 170], f16)
            nc.scalar.dma_start(out=c16, in_=cpk16.ap())
            w2gv_sb = c16[:, 0:18]
            gu16_sb = c16[:, 18:42]
            ones16_sb = c16[:, 42:170]
            cp = const.tile([128, 128], f32)
            nc.scalar.dma_start(out=cp, in_=cpack.ap())
            ident = cp[:, 0:128]
            oneh_sb = const.tile([12, 12 * 128], f16)
            nc.scalar.dma_start(out=oneh_sb, in_=oneh.ap())
            w2t_sb = const.tile([128, FCH, H], f16)
            if has_b2:
                b2bc_sb = const.tile([128, D, H], f32)
                nc.scalar.dma_start(out=b2bc_sb, in_=b2bc.ap())

            xt_first = xtp.tile([128, KCH, T], dt1, tag="xt")
            nc.sync.dma_start(out=xt_first, in_=xt_v[:, :, 0:T])

            def stage_a(mt, xt_pre=None, xq=None):
                """xT + x loads, stats -> r8 (= SC/s) per sub-tile."""
                xq = xq or nc.sync
                st = {}
                if xt_pre is not None:
                    xt_t = xt_pre
                else:
                    xt_t = xtp.tile([128, KCH, T], dt1, tag="xt")
                    nc.sync.dma_start(out=xt_t,
                                      in_=xt_v[:, :, mt * T:(mt + 1) * T])
                x_t = xp.tile([128, NSUB, H], f32, tag="x")
                for ss in range(NSUB):
                    xq.dma_start(out=x_t[:, ss, :], in_=x_mt[mt][:, ss, :])

                # rmu[:, ss, :] = [r8=SC/s, s]
                rmu = xtp.tile([128, NSUB, 2], f32, tag="rmu")
                srow = xtp.tile([1, T], f16, tag="srow") if has_b1e else None
                for ss in range(NSUB):
                    xs = x_t[:, ss, :]
                    stt_ = smalls.tile([128, 2, 6], f32, tag="bnst")
                    nc.vector.bn_stats(out=stt_[:, 0, :], in_=xs[:, 0:512])
                    nc.vector.bn_stats(out=stt_[:, 1, :], in_=xs[:, 512:1024])
                    mv = smalls.tile([128, 2], f32, tag="mv")
                    nc.vector.bn_aggr(out=mv, in_=stt_)
                    # s = sqrt(var*H/(H-1)) + eps; r8 = SC/s
                    nc.scalar.activation(out=rmu[:, ss, 1:2], in_=mv[:, 1:2],
                                         func=AF.Sqrt, scale=float(H) / (H - 1))
                    nc.vector.tensor_scalar_add(rmu[:, ss, 1:2],
                                                rmu[:, ss, 1:2], EPS)
                    nc.vector.reciprocal(rmu[:, ss, 0:1], rmu[:, ss, 1:2])
                    if SC != 1.0:
                        nc.vector.tensor_scalar_mul(rmu[:, ss, 0:1],
                                                    rmu[:, ss, 0:1], SC)
                    if has_b1e:
                        ptr = ps_sm.tile([1, 128], f32, tag="sm")
                        nc.tensor.transpose(ptr, rmu[:, ss, 1:2], ident)
                        nc.scalar.activation(
                            out=srow[:, ss * 128:(ss + 1) * 128],
                            in_=ptr, func=AF.Copy)
                st.update(xt_t=xt_t, x_t=x_t, rmu=rmu, srow=srow)
                return st

            def b_murow(mt, st):
                """mu row [1,T] from the PE: ones^T @ xT / H."""
                xt_t = st["xt_t"]
                pmu = ps_m1.tile([128, T], f32, tag="m1")
                if fp8:
                    for k in range(KCH // 2):
                        nc.tensor.matmul(pmu, ones8_sb,
                                         xt_t[:, 2 * k:2 * k + 2, :],
                                         start=(k == 0), stop=(k == 3),
                                         perf_mode=DR)
                else:
                    for k in range(KCH):
                        nc.tensor.matmul(pmu, ones16_sb,
                                         xt_t[:, k, :],
                                         start=(k == 0), stop=(k == KCH - 1))
                murow = xtp.tile([1, T], f16, tag="murow")
                nc.scalar.activation(out=murow, in_=pmu[0:1, :], func=AF.Copy,
                                     scale=1.0 / H)
                st["murow"] = murow

            def b_chunk(mt, st, c):
                """One M1 output chunk: DR/f16 matmuls + rank-1 + relu."""
                xt_t, murow = st["xt_t"], st["murow"]
                if c == 0:
                    mid_t = midp.tile([128, FCH, T], f16, tag="mid")
                    st["mid"] = mid_t
                mid = st["mid"]
                csl = slice(c * 128, (c + 1) * 128)
                p1 = ps_m1.tile([128, T], f32, tag="m1")
                if fp8:
                    for k in range(KCH // 2):
                        nc.tensor.matmul(
                            p1, w1hi_sb[:, 2 * k:2 * k + 2, csl],
                            xt_t[:, 2 * k:2 * k + 2, :],
                            start=(k == 0), stop=False, perf_mode=DR)
                    for k in range(KCH // 2):
                        nc.tensor.matmul(
                            p1, w1lo_sb[:, 2 * k:2 * k + 2, csl],
                            xt_t[:, 2 * k:2 * k + 2, :],
                            start=False, stop=False, perf_mode=DR)
                else:
                    for k in range(KCH):
                        nc.tensor.matmul(
                            p1, w1hi_sb[:, k, csl], xt_t[:, k, :],
                            start=(k == 0), stop=False)
                w1s = r16[:, RO_W1S + c * 128:RO_W1S + (c + 1) * 128]
                nc.tensor.matmul(p1, w1s, murow,
                                 start=False, stop=not has_b1e)
                if has_b1e:
                    b1s = r16[:, RO_B1E + c * 128:RO_B1E + (c + 1) * 128]
                    nc.tensor.matmul(p1, b1s, st["srow"],
                                     start=False, stop=True)
                nc.scalar.activation(out=mid[:, c, :], in_=p1,
                                     func=AF.Relu, scale=ISC)

            def cg_ss(mt, st, ss):
                """Gate logits/sigmoid/rg for one sub-tile."""
                xt_t, mid, rmu = st["xt_t"], st["mid"], st["rmu"]
                if ss == 0:
                    rgall_t = gsm.tile([128, NSUB * D], f32, tag="rgall")
                    st["rgall"] = rgall_t
                    if has_b2:
                        gall_t = gsm.tile([128, NSUB * D], f32, tag="gall")
                        st["gall"] = gall_t
                tsl = slice(ss * 128, (ss + 1) * 128)
                pgx = ps_sm.tile([128, 3], f32, tag="sm")
                if fp8:
                    for k in range(KCH // 2):
                        nc.tensor.matmul(
                            pgx, xt_t[:, 2 * k:2 * k + 2, tsl],
                            gu8_sb[:, 2 * k:2 * k + 2, :],
                            start=(k == 0), stop=False, perf_mode=DR)
                else:
                    for k in range(KCH):
                        nc.tensor.matmul(
                            pgx, xt_t[:, k, tsl],
                            gu16_sb[:, k * D:(k + 1) * D],
                            start=(k == 0), stop=False)
                nc.tensor.matmul(pgx, r16[:, RO_ONES:RO_ONES + 128],
                                 r16[:, RO_GB:RO_GB + 3],
                                 start=False, stop=True)
                pgv = ps_sm.tile([128, 3], f32, tag="sm")
                for c in range(FCH):
                    nc.tensor.matmul(pgv, mid[:, c, tsl],
                                     w2gv_sb[:, c * D:(c + 1) * D],
                                     start=(c == 0), stop=(c == FCH - 1))
                gx = gsm.tile([128, 3], f32, tag="gx")
                nc.scalar.activation(out=gx, in_=pgx, func=AF.Copy)
                z = gsm.tile([128, 3], f32, tag="z")
                nc.vector.scalar_tensor_tensor(
                    out=z, in0=pgv, scalar=rmu[:, ss, 0:1], in1=gx,
                    op0=ALU.mult, op1=ALU.add)
                g3 = gsm.tile([128, 3], f32, tag="g3")
                nc.scalar.activation(out=g3, in_=z, func=AF.Sigmoid,
                                     scale=ISC)
                nc.gpsimd.tensor_scalar(
                    out=st["rgall"][:, ss * D:(ss + 1) * D], in0=g3,
                    scalar1=rmu[:, ss, 0:1], scalar2=ISC,
                    op0=ALU.mult, op1=ALU.mult)
                if has_b2:
                    nc.gpsimd.tensor_scalar(
                        out=st["gall"][:, ss * D:(ss + 1) * D], in0=g3,
                        scalar1=1.0, scalar2=None, op0=ALU.mult)

            def cg_fin(mt, st):
                """Transpose rg [128,12] -> [12,128] f16."""
                ptr = ps_sm.tile([NSUB * D, 128], f32, tag="sm")
                nc.tensor.transpose(ptr, st["rgall"], ident)
                rgT = gsm.tile([NSUB * D, 128], f16, tag="rgT")
                nc.scalar.activation(out=rgT, in_=ptr, func=AF.Copy)
                st["rgT"] = rgT

            def emit_bcast(mt, st, ss):
                """rg rows for sub-tile ss -> SBUF; gmid = mid*rg (Pool)."""
                mid, rgT = st["mid"], st["rgT"]
                tsl = slice(ss * 128, (ss + 1) * 128)
                gmid = midp.tile([128, FCH, 128], f16, tag=f"gmid{ss}")
                for d in range(D):
                    j = ss * D + d
                    pb = ps_bc.tile([128, 128], f32, tag="bc")
                    nc.tensor.matmul(pb, oneh_sb[:, j * 128:(j + 1) * 128],
                                     rgT, start=True, stop=True)
                    pbs = gsm.tile([128, 128], f16, tag="pbs")
                    nc.scalar.activation(out=pbs, in_=pb, func=AF.Copy)
                    for fh in range(2):
                        c = d * 2 + fh
                        nc.gpsimd.tensor_mul(gmid[:, c, :], mid[:, c, tsl],
                                             pbs)
                st[f"gmid{ss}"] = gmid

            def emit_m2(mt, st, ss):
                """M2 for sub-tile ss + residual combine + out DMA."""
                gmid, x_t = st[f"gmid{ss}"], st["x_t"]
                out_sb = outp.tile([128, H], f32, tag="osb")
                for nch in range(NCH):
                    hsl = slice(nch * 512, (nch + 1) * 512)
                    po = ps_m2.tile([128, 512], f32, tag="m2")
                    for c in range(FCH):
                        nc.tensor.matmul(po, gmid[:, c, :], w2t_sb[:, c, hsl],
                                         start=(c == 0), stop=(c == FCH - 1))
                    nc.vector.scalar_tensor_tensor(
                        out=out_sb[:, hsl], in0=x_t[:, ss, hsl],
                        scalar=2.0, in1=po, op0=ALU.mult, op1=ALU.add)
                    if has_b2:
                        for d in range(D):
                            nc.vector.scalar_tensor_tensor(
                                out=out_sb[:, hsl], in0=b2bc_sb[:, d, hsl],
                                scalar=st["gall"][:, ss * D + d:ss * D + d + 1],
                                in1=out_sb[:, hsl], op0=ALU.mult, op1=ALU.add)
                    nc.sync.dma_start(out=out_mt[mt][:, ss, hsl],
                                      in_=out_sb[:, hsl])

            def cg_interleaved(mt, st, st_next):
                """Gate chain for mt with next tile's M1 chunks as PE fill;
                the first two rg-broadcasts ride along at the end."""
                for ss in range(NSUB):
                    cg_ss(mt, st, ss)
                    b_chunk(mt + 1, st_next, ss)
                cg_fin(mt, st)
                emit_bcast(mt, st, 0)
                b_chunk(mt + 1, st_next, 4)
                emit_bcast(mt, st, 1)
                b_chunk(mt + 1, st_next, 5)

            def cbd_rest(mt, st):
                """Remaining broadcasts + all M2 (one sub-tile behind)."""
                emit_bcast(mt, st, 2)
                emit_m2(mt, st, 0)
                emit_bcast(mt, st, 3)
                emit_m2(mt, st, 1)
                emit_m2(mt, st, 2)
                emit_m2(mt, st, 3)

            # ---- software pipeline over macro-tiles ----
            S = [None] * NMT
            S[0] = stage_a(0, xt_pre=xt_first, xq=nc.gpsimd)
            nc.gpsimd.dma_start(out=w2t_sb[:, 0:3, :], in_=w2t.ap()[:, 0:3, :])
            nc.gpsimd.dma_start(out=w2t_sb[:, 3:6, :], in_=w2t.ap()[:, 3:6, :])
            b_murow(0, S[0])
            for c in range(FCH):
                b_chunk(0, S[0], c)
            S[1] = stage_a(1)
            b_murow(1, S[1])
            cg_interleaved(0, S[0], S[1])
            cbd_rest(0, S[0])
            S[2] = stage_a(2)
            b_murow(2, S[2])
            cg_interleaved(1, S[1], S[2])
            cbd_rest(1, S[1])
            S[3] = stage_a(3)
            b_murow(3, S[3])
            cg_interleaved(2, S[2], S[3])
            emit_bcast(2, S[2], 2)
            emit_bcast(2, S[2], 3)
            # tail: mt3 gate chain filled with mt2's M2 work
            cg_ss(3, S[3], 0)
            emit_m2(2, S[2], 0)
            cg_ss(3, S[3], 1)
            emit_m2(2, S[2], 1)
            cg_ss(3, S[3], 2)
            emit_m2(2, S[2], 2)
            cg_ss(3, S[3], 3)
            emit_m2(2, S[2], 3)
            cg_fin(3, S[3])
            emit_bcast(3, S[3], 0)
            emit_bcast(3, S[3], 1)
            cbd_rest(3, S[3])

    _split_multiwaits(nc)
    return nc


_built = {}


def _get_nc(has_b1e, has_b2, mode=None):
    key = (has_b1e, has_b2, mode or MODE)
    if key not in _built:
        _built[key] = _build(has_b1e, has_b2, mode)
    return _built[key]


last_results = None


def kernel(x, ln_g, ln_b, W1, b1, W2, b2, gu, gv, gb):
    import ml_dtypes
    E4 = ml_dtypes.float8_e4m3

    x = np.asarray(x, dtype=np.float32)
    ln_g = np.asarray(ln_g, dtype=np.float32)
    ln_b = np.asarray(ln_b, dtype=np.float32)
    W1 = np.asarray(W1, dtype=np.float32)
    b1 = np.asarray(b1, dtype=np.float32)
    W2 = np.asarray(W2, dtype=np.float32)
    b2 = np.asarray(b2, dtype=np.float32)
    gu = np.asarray(gu, dtype=np.float32)
    gv = np.asarray(gv, dtype=np.float32)
    gb = np.asarray(gb, dtype=np.float32)

    fp8 = MODE == "fp8"
    ndt = E4 if fp8 else np.float16
    SC = 8.0 if fp8 else 1.0

    # ---- host packing (weights/layout only) ----
    W1g = np.transpose(W1, (0, 2, 1)) * ln_g[:, :, None]       # [D,H,F]
    b1e = b1 + np.einsum('dfh,dh->df', W1, ln_b)               # [D,F]
    w2gv = np.einsum('dh,dhf->df', gv, W2)                     # [D,F]
    gb_eff = gb + np.einsum('dh,dh->d', gv, b2)                # [D]
    has_b1e = bool(np.any(b1e != 0.0))
    has_b2 = bool(np.any(b2 != 0.0))

    # M1 lhsT [128, KCH, DF], chunk c=(d, fh)
    w1full = np.zeros((128, KCH, DF), np.float32)
    for c in range(FCH):
        d, fh = c // 2, c % 2
        w1full[:, :, c * 128:(c + 1) * 128] = (
            SC * W1g[d].reshape(KCH, 128, F)[:, :, fh * 128:(fh + 1) * 128]
            .transpose(1, 0, 2))
    w1hi_in = w1full.astype(ndt)
    if fp8:
        w1lo_in = (w1full - w1hi_in.astype(np.float32)).astype(E4)
    # M2 rhs [128, FCH, H]: w2t[p, c, h] = W2[d, h, fh*128+p]
    w2t_in = np.zeros((128, FCH, H), np.float16)
    for c in range(FCH):
        d, fh = c // 2, c % 2
        w2t_in[:, c, :] = W2[d, :, fh * 128:(fh + 1) * 128].T
    # block-diag w2gv [128,18] + gu chunks [128,24] + ones8 [128,8]
    c16_in = np.zeros((128, 170), np.float16)
    for c in range(FCH):
        d, fh = c // 2, c % 2
        c16_in[:, c * D + d] = w2gv[d, fh * 128:(fh + 1) * 128]
    if not fp8:
        for k in range(KCH):
            c16_in[:, 18 + k * D:18 + (k + 1) * D] = \
                (SC * gu[:, k * 128:(k + 1) * 128]).T
    c16_in[:, 42:170] = 1.0
    row16_in = np.zeros((1, RO_END), np.float16)
    row16_in[0, RO_ONES:RO_ONES + 128] = 1.0
    row16_in[0, RO_GB:RO_GB + D] = SC * gb_eff
    row16_in[0, RO_W1S:RO_W1S + DF] = -w1full.sum(axis=(0, 1))
    if has_b1e:
        b1e_pack = np.zeros(DF, np.float32)
        for c in range(FCH):
            d, fh = c // 2, c % 2
            b1e_pack[c * 128:(c + 1) * 128] = \
                SC * b1e[d, fh * 128:(fh + 1) * 128]
        row16_in[0, RO_B1E:RO_B1E + DF] = b1e_pack
    oneh_in = np.zeros((12, 12 * 128), np.float16)
    for j in range(12):
        oneh_in[j, j * 128:(j + 1) * 128] = 1.0
    cpack_in = np.eye(128, dtype=np.float32)

    common = {
        "w1hi": w1hi_in, "w2t": w2t_in, "cpack": cpack_in,
        "cpk16": c16_in, "row16": row16_in, "oneh": oneh_in,
    }
    if fp8:
        common["w1lo"] = w1lo_in
        cpk8_in = np.zeros((128, 280), np.float32)
        for k in range(KCH):
            cpk8_in[:, k * D:(k + 1) * D] = \
                (SC * gu[:, k * 128:(k + 1) * 128]).T
        cpk8_in[:, 24:280] = 1.0
        common["cpk8"] = cpk8_in.astype(E4)
    if has_b2:
        common["b2bc"] = np.broadcast_to(
            b2[None, :, :], (128, D, H)).astype(np.float32).copy()

    nc = _get_nc(has_b1e, has_b2)

    in_maps = []
    for c in range(B):
        m = dict(common, xin=np.ascontiguousarray(x[c]))
        m["xt"] = np.ascontiguousarray(x[c].T).astype(ndt)
        in_maps.append(m)
    res = run_bass_kernel_spmd(nc, in_maps, core_ids=list(range(B)))
    global last_results
    last_results = res
    return np.stack([res.results[c]["out"] for c in range(B)])


# revision 13
# speedup vs baseline: 1.0950x; 1.0271x over previous
"""Trainium2 Bass kernel for nn_MixtureOfAdapter (moe_routing), v3.

Math (per token, H=1024, F=256, D=3 domains; grading inputs have
ln_g=1, ln_b=0, b1=0, b2=0, gb=0):
    mu, sd (ddof=1) over H;  s = sd + eps;  xn = (x - mu)/s
    mid_d = relu(W1g_d xn + b1e_d);  a_d = W2_d mid_d + b2_d
    gate_d = sigmoid(gu_d.x + gv_d.a_d + gb_d)
    out = 2x + sum_d gate_d * a_d

Kernel strategy (8 cores, data-parallel over batch B=8):
  - Ship TWO copies of x per core: natural f32 [L,H] (stats via
    bn_stats + final residual) and a host-transposed [H,L] copy in a
    compact dtype (f16, or fp8e4 in fp8 mode) that feeds all matmuls.
    No PE transposes of x, no on-device centering.
  - M1 runs on UNCENTERED xT; centering folds into a rank-1
    correction: out1 = W1q^T xq - colsum(W1q)*mu.  mu itself comes
    from the PE (ones^T @ xT / H), so M1 never waits on the natural-x
    DMA or the stats chain.  relu(out1) = s*mid ("mid_s"), f16.
  - fp8 mode: W1 split hi+lo fp8e4 (noise-free weights), xT single
    fp8; DoubleRow matmuls (0.5 cycles/row, 2 k-chunks/pass) = 4x
    f32r rate.  SC=8 on (W1, gu, gb); relu/sigmoid absorb 1/SC.
  - Gates in [token, domain] layout: pgux[t,d] + gb rank-1; pgv[t,d]
    from mid chunks; z = pgv*r8 + pgux (DVE); gate = sigmoid(z/SC);
    rg = gate*r -> [128,12] -> one PE transpose -> one-hot broadcast
    per (ss,d) -> Act copy to SBUF -> gmid = mid*rg on Pool (f16).
  - M2 (f16) accumulates all domains into one PSUM per (ss, half);
    out = 2x + pout via one DVE scalar_tensor_tensor per half.
  - Emission interleaves next tile's M1 chunks into the gate chain so
    the PE FIFO never drains behind DVE/Act/Pool latency.
"""

import numpy as np

import concourse.bass as bass
import concourse.mybir as mybir
import concourse.tile as tile
from concourse.bass_utils import run_bass_kernel_spmd

B, L, H, F, D = 8, 2048, 1024, 256, 3
EPS = 1e-6
T = 512                 # tokens per macro-tile
NSUB = T // 128         # 4 sub-tiles of 128 tokens
NMT = L // T            # 4 macro-tiles per core
KCH = H // 128          # 8 k-chunks over H
FCH = (D * F) // 128    # 6 chunks over stacked (domain, F)
NCH = H // 512          # 2 output column chunks
DF = D * F

f32 = mybir.dt.float32
f16 = mybir.dt.float16
f8 = mybir.dt.float8e4
AF = mybir.ActivationFunctionType
ALU = mybir.AluOpType
DR = mybir.MatmulPerfMode.DoubleRow

MODE = "fp8"            # "fp8" (DoubleRow M1) or "f16"

# row16 layout (single-partition f16 consts)
RO_ONES = 0             # [0:128] ones
RO_GB = 128             # [128:131] SC*gb_eff
RO_W1S = 131            # [131:899] -SC*colsum(W1g)
RO_B1E = 899            # [899:1667] SC*b1e
RO_END = 1667


def _split_multiwaits(nc):
    """This walrus build allows 1 sync-wait per instruction (2 for
    EventSemaphore); Tile can attach more.  Move extras onto preceding
    same-engine NoOps (engine queues are FIFO, so semantics identical)."""
    for fn in nc.m.functions:
        for bb in fn.blocks:
            new = []
            changed = False
            for inst in bb.instructions:
                si = inst.sync_info
                cap = 2 if isinstance(inst, mybir.InstEventSemaphore) else 1
                if si is not None and len(si.on_wait) > cap:
                    waits = list(si.on_wait)
                    extra, kept = waits[:-cap], waits[-cap:]
                    for j, w in enumerate(extra):
                        new.append(mybir.InstNoOp(
                            name=f"{inst.name}-wsplit{j}",
                            engine=inst.engine,
                            sync_info=mybir.SyncInfo(on_wait=[w], on_update=[]),
                            ins=[], outs=[],
                        ))
                    inst.sync_info = mybir.SyncInfo(
                        on_wait=kept, on_update=list(si.on_update))
                    changed = True
                new.append(inst)
            if changed:
                bb.instructions = new


def _build(has_b1e: bool, has_b2: bool, mode: str = None):
    mode = mode or MODE
    fp8 = mode == "fp8"
    dt1 = f8 if fp8 else f16
    SC = 8.0 if fp8 else 1.0
    ISC = 1.0 / SC

    nc = bass.Bass(target_bir_lowering=False)

    xin = nc.dram_tensor("xin", [L, H], f32, kind="ExternalInput")
    xt = nc.dram_tensor("xt", [H, L], dt1, kind="ExternalInput")
    w1hi = nc.dram_tensor("w1hi", [128, KCH, DF], dt1, kind="ExternalInput")
    if fp8:
        w1lo = nc.dram_tensor("w1lo", [128, KCH, DF], f8, kind="ExternalInput")
    w2t = nc.dram_tensor("w2t", [128, FCH, H], f16, kind="ExternalInput")
    cpack = nc.dram_tensor("cpack", [128, 128], f32, kind="ExternalInput")
    cpk16 = nc.dram_tensor("cpk16", [128, 170], f16, kind="ExternalInput")
    row16 = nc.dram_tensor("row16", [1, RO_END], f16, kind="ExternalInput")
    oneh = nc.dram_tensor("oneh", [12, 12 * 128], f16, kind="ExternalInput")
    if fp8:
        cpk8 = nc.dram_tensor("cpk8", [128, 280], f8, kind="ExternalInput")
    if has_b2:
        b2bc = nc.dram_tensor("b2bc", [128, D, H], f32, kind="ExternalInput")
    out = nc.dram_tensor("out", [L, H], f32, kind="ExternalOutput")

    x_mt = xin.ap().rearrange("(m s p) h -> m p s h", p=128, s=NSUB)
    xt_v = xt.ap().rearrange("(k p) l -> p k l", p=128)
    out_mt = out.ap().rearrange("(m s p) h -> m p s h", p=128, s=NSUB)

    with tile.TileContext(nc) as tc:
        with (
            tc.tile_pool(name="const", bufs=1) as const,
            tc.tile_pool(name="xp", bufs=2) as xp,
            tc.tile_pool(name="xtp", bufs=2) as xtp,
            tc.tile_pool(name="midp", bufs=2) as midp,
            tc.tile_pool(name="outp", bufs=2) as outp,
            tc.tile_pool(name="smalls", bufs=3) as smalls,
            tc.tile_pool(name="gsm", bufs=2) as gsm,
            tc.tile_pool(name="ps_m1", bufs=2, space="PSUM") as ps_m1,
            tc.tile_pool(name="ps_m2", bufs=2, space="PSUM") as ps_m2,
            tc.tile_pool(name="ps_sm", bufs=2, space="PSUM") as ps_sm,
            tc.tile_pool(name="ps_bc", bufs=2, space="PSUM") as ps_bc,
        ):
            # ---- startup: sync carries xt0 + M1/M2 weights in use
            # order; scalar carries small consts + mt0's natural x ----
            xt_first = xtp.tile([128, KCH, T], dt1, tag="xt")
            nc.sync.dma_start(out=xt_first, in_=xt_v[:, :, 0:T])
            if fp8:
                c8 = const.tile([128, 280], f8)
                nc.sync.dma_start(out=c8, in_=cpk8.ap())
                gu8_sb = c8[:, 0:24].rearrange("p (k d) -> p k d", d=D)
                ones8_sb = c8[:, 24:280].rearrange("p (t o) -> p t o", t=2)
            w1hi_sb = const.tile([128, KCH, DF], dt1)
            nc.sync.dma_start(out=w1hi_sb, in_=w1hi.ap())
            if fp8:
                w1lo_sb = const.tile([128, KCH, DF], f8)
                nc.sync.dma_start(out=w1lo_sb, in_=w1lo.ap())
            r16 = const.tile([1, RO_END], f16)
            nc.sync.dma_start(out=r16, in_=row16.ap())
            w2t_sb = const.tile([128, FCH, H], f16)
            nc.sync.dma_start(out=w2t_sb[:, 0:3, :], in_=w2t.ap()[:, 0:3, :])
            nc.sync.dma_start(out=w2t_sb[:, 3:6, :], in_=w2t.ap()[:, 3:6, :])
            c16 = const.tile([128, 170], f16)
            nc.scalar.dma_start(out=c16, in_=cpk16.ap())
            w2gv_sb = c16[:, 0:18]
            gu16_sb = c16[:, 18:42]
            ones16_sb = c16[:, 42:170]
            cp = const.tile([128, 128], f32)
            nc.scalar.dma_start(out=cp, in_=cpack.ap())
            ident = cp[:, 0:128]
            oneh_sb = const.tile([12, 12 * 128], f16)
            nc.scalar.dma_start(out=oneh_sb, in_=oneh.ap())
            if has_b2:
                b2bc_sb = const.tile([128, D, H], f32)
                nc.scalar.dma_start(out=b2bc_sb, in_=b2bc.ap())

            def stage_a(mt, xt_pre=None, xq=None):
                """xT + x loads, stats -> r8 (= SC/s) per sub-tile."""
                xq = xq or nc.sync
                st = {}
                if xt_pre is not None:
                    xt_t = xt_pre
                else:
                    xt_t = xtp.tile([128, KCH, T], dt1, tag="xt")
                    nc.sync.dma_start(out=xt_t,
                                      in_=xt_v[:, :, mt * T:(mt + 1) * T])
                x_t = xp.tile([128, NSUB, H], f32, tag="x")
                for ss in range(NSUB):
                    xq.dma_start(out=x_t[:, ss, :], in_=x_mt[mt][:, ss, :])

                # rmu[:, ss, :] = [r8=SC/s, s]
                rmu = xtp.tile([128, NSUB, 2], f32, tag="rmu")
                srow = xtp.tile([1, T], f16, tag="srow") if has_b1e else None
                for ss in range(NSUB):
                    xs = x_t[:, ss, :]
                    stt_ = smalls.tile([128, 2, 6], f32, tag="bnst")
                    nc.vector.bn_stats(out=stt_[:, 0, :], in_=xs[:, 0:512])
                    nc.vector.bn_stats(out=stt_[:, 1, :], in_=xs[:, 512:1024])
                    mv = smalls.tile([128, 2], f32, tag="mv")
                    nc.vector.bn_aggr(out=mv, in_=stt_)
                    # s = sqrt(var*H/(H-1)) + eps; r8 = SC/s
                    nc.scalar.activation(out=rmu[:, ss, 1:2], in_=mv[:, 1:2],
                                         func=AF.Sqrt, scale=float(H) / (H - 1))
                    nc.vector.tensor_scalar_add(rmu[:, ss, 1:2],
                                                rmu[:, ss, 1:2], EPS)
                    nc.vector.reciprocal(rmu[:, ss, 0:1], rmu[:, ss, 1:2])
                    if SC != 1.0:
                        nc.vector.tensor_scalar_mul(rmu[:, ss, 0:1],
                                                    rmu[:, ss, 0:1], SC)
                    if has_b1e:
                        ptr = ps_sm.tile([1, 128], f32, tag="sm")
                        nc.tensor.transpose(ptr, rmu[:, ss, 1:2], ident)
                        nc.scalar.activation(
                            out=srow[:, ss * 128:(ss + 1) * 128],
                            in_=ptr, func=AF.Copy)
                st.update(xt_t=xt_t, x_t=x_t, rmu=rmu, srow=srow)
                return st

            def b_murow(mt, st):
                """mu row [1,T] from the PE: ones^T @ xT / H."""
                xt_t = st["xt_t"]
                pmu = ps_m1.tile([128, T], f32, tag="m1")
                if fp8:
                    for k in range(KCH // 2):
                        nc.tensor.matmul(pmu, ones8_sb,
                                         xt_t[:, 2 * k:2 * k + 2, :],
                                         start=(k == 0), stop=(k == 3),
                                         perf_mode=DR)
                else:
                    for k in range(KCH):
                        nc.tensor.matmul(pmu, ones16_sb,
                                         xt_t[:, k, :],
                                         start=(k == 0), stop=(k == KCH - 1))
                murow = xtp.tile([1, T], f16, tag="murow")
                nc.scalar.activation(out=murow, in_=pmu[0:1, :], func=AF.Copy,
                                     scale=1.0 / H)
                st["murow"] = murow

            def b_chunk(mt, st, c):
                """One M1 output chunk: DR/f16 matmuls + rank-1 + relu."""
                xt_t, murow = st["xt_t"], st["murow"]
                if c == 0:
                    mid_t = midp.tile([128, FCH, T], f16, tag="mid")
                    st["mid"] = mid_t
                mid = st["mid"]
                csl = slice(c * 128, (c + 1) * 128)
                p1 = ps_m1.tile([128, T], f32, tag="m1")
                if fp8:
                    for k in range(KCH // 2):
                        nc.tensor.matmul(
                            p1, w1hi_sb[:, 2 * k:2 * k + 2, csl],
                            xt_t[:, 2 * k:2 * k + 2, :],
                            start=(k == 0), stop=False, perf_mode=DR)
                    for k in range(KCH // 2):
                        nc.tensor.matmul(
                            p1, w1lo_sb[:, 2 * k:2 * k + 2, csl],
                            xt_t[:, 2 * k:2 * k + 2, :],
                            start=False, stop=False, perf_mode=DR)
                else:
                    for k in range(KCH):
                        nc.tensor.matmul(
                            p1, w1hi_sb[:, k, csl], xt_t[:, k, :],
                            start=(k == 0), stop=False)
                w1s = r16[:, RO_W1S + c * 128:RO_W1S + (c + 1) * 128]
                nc.tensor.matmul(p1, w1s, murow,
                                 start=False, stop=not has_b1e)
                if has_b1e:
                    b1s = r16[:, RO_B1E + c * 128:RO_B1E + (c + 1) * 128]
                    nc.tensor.matmul(p1, b1s, st["srow"],
                                     start=False, stop=True)
                nc.scalar.activation(out=mid[:, c, :], in_=p1,
                                     func=AF.Relu, scale=ISC)

            def cg_ss(mt, st, ss):
                """Gate logits/sigmoid/rg for one sub-tile."""
                xt_t, mid, rmu = st["xt_t"], st["mid"], st["rmu"]
                if ss == 0:
                    rgall_t = gsm.tile([128, NSUB * D], f32, tag="rgall")
                    st["rgall"] = rgall_t
                    if has_b2:
                        gall_t = gsm.tile([128, NSUB * D], f32, tag="gall")
                        st["gall"] = gall_t
                tsl = slice(ss * 128, (ss + 1) * 128)
                pgx = ps_sm.tile([128, 3], f32, tag="sm")
                if fp8:
                    for k in range(KCH // 2):
                        nc.tensor.matmul(
                            pgx, xt_t[:, 2 * k:2 * k + 2, tsl],
                            gu8_sb[:, 2 * k:2 * k + 2, :],
                            start=(k == 0), stop=False, perf_mode=DR)
                else:
                    for k in range(KCH):
                        nc.tensor.matmul(
                            pgx, xt_t[:, k, tsl],
                            gu16_sb[:, k * D:(k + 1) * D],
                            start=(k == 0), stop=False)
                nc.tensor.matmul(pgx, r16[:, RO_ONES:RO_ONES + 128],
                                 r16[:, RO_GB:RO_GB + 3],
                                 start=False, stop=True)
                pgv = ps_sm.tile([128, 3], f32, tag="sm")
                for c in range(FCH):
                    nc.tensor.matmul(pgv, mid[:, c, tsl],
                                     w2gv_sb[:, c * D:(c + 1) * D],
                                     start=(c == 0), stop=(c == FCH - 1))
                gx = gsm.tile([128, 3], f32, tag="gx")
                nc.scalar.activation(out=gx, in_=pgx, func=AF.Copy)
                z = gsm.tile([128, 3], f32, tag="z")
                nc.vector.scalar_tensor_tensor(
                    out=z, in0=pgv, scalar=rmu[:, ss, 0:1], in1=gx,
                    op0=ALU.mult, op1=ALU.add)
                g3 = gsm.tile([128, 3], f32, tag="g3")
                nc.scalar.activation(out=g3, in_=z, func=AF.Sigmoid,
                                     scale=ISC)
                nc.gpsimd.tensor_scalar(
                    out=st["rgall"][:, ss * D:(ss + 1) * D], in0=g3,
                    scalar1=rmu[:, ss, 0:1], scalar2=ISC,
                    op0=ALU.mult, op1=ALU.mult)
                if has_b2:
                    nc.gpsimd.tensor_scalar(
                        out=st["gall"][:, ss * D:(ss + 1) * D], in0=g3,
                        scalar1=1.0, scalar2=None, op0=ALU.mult)

            def cg_fin(mt, st):
                """Transpose rg [128,12] -> [12,128] f16."""
                ptr = ps_sm.tile([NSUB * D, 128], f32, tag="sm")
                nc.tensor.transpose(ptr, st["rgall"], ident)
                rgT = gsm.tile([NSUB * D, 128], f16, tag="rgT")
                nc.scalar.activation(out=rgT, in_=ptr, func=AF.Copy)
                st["rgT"] = rgT

            def emit_bcast(mt, st, ss):
                """rg rows for sub-tile ss -> SBUF; gmid = mid*rg (Pool)."""
                mid, rgT = st["mid"], st["rgT"]
                tsl = slice(ss * 128, (ss + 1) * 128)
                gmid = midp.tile([128, FCH, 128], f16, tag=f"gmid{ss}")
                for d in range(D):
                    j = ss * D + d
                    pb = ps_bc.tile([128, 128], f32, tag="bc")
                    nc.tensor.matmul(pb, oneh_sb[:, j * 128:(j + 1) * 128],
                                     rgT, start=True, stop=True)
                    pbs = gsm.tile([128, 128], f16, tag="pbs")
                    nc.scalar.activation(out=pbs, in_=pb, func=AF.Copy)
                    nc.vector.tensor_mul(gmid[:, d * 2, :],
                                         mid[:, d * 2, tsl], pbs)
                    nc.gpsimd.tensor_mul(gmid[:, d * 2 + 1, :],
                                         mid[:, d * 2 + 1, tsl], pbs)
                st[f"gmid{ss}"] = gmid

            def emit_m2(mt, st, ss):
                """M2 for sub-tile ss + residual combine + out DMA."""
                gmid, x_t = st[f"gmid{ss}"], st["x_t"]
                out_sb = outp.tile([128, H], f32, tag="osb")
                for nch in range(NCH):
                    hsl = slice(nch * 512, (nch + 1) * 512)
                    po = ps_m2.tile([128, 512], f32, tag="m2")
                    for c in range(FCH):
                        nc.tensor.matmul(po, gmid[:, c, :], w2t_sb[:, c, hsl],
                                         start=(c == 0), stop=(c == FCH - 1))
                    nc.vector.scalar_tensor_tensor(
                        out=out_sb[:, hsl], in0=x_t[:, ss, hsl],
                        scalar=2.0, in1=po, op0=ALU.mult, op1=ALU.add)
                    if has_b2:
                        for d in range(D):
                            nc.vector.scalar_tensor_tensor(
                                out=out_sb[:, hsl], in0=b2bc_sb[:, d, hsl],
                                scalar=st["gall"][:, ss * D + d:ss * D + d + 1],
                                in1=out_sb[:, hsl], op0=ALU.mult, op1=ALU.add)
                    nc.sync.dma_start(out=out_mt[mt][:, ss, hsl],
                                      in_=out_sb[:, hsl])

            def cg_interleaved(mt, st, st_next):
                """Gate chain for mt with next tile's M1 chunks as PE fill;
                the first two rg-broadcasts ride along at the end."""
                for ss in range(NSUB):
                    cg_ss(mt, st, ss)
                    b_chunk(mt + 1, st_next, ss)
                cg_fin(mt, st)
                emit_bcast(mt, st, 0)
                b_chunk(mt + 1, st_next, 4)
                emit_bcast(mt, st, 1)
                b_chunk(mt + 1, st_next, 5)

            def cbd_rest(mt, st):
                """Remaining broadcasts + all M2 (one sub-tile behind)."""
                emit_bcast(mt, st, 2)
                emit_m2(mt, st, 0)
                emit_bcast(mt, st, 3)
                emit_m2(mt, st, 1)
                emit_m2(mt, st, 2)
                emit_m2(mt, st, 3)

            # ---- software pipeline over macro-tiles ----
            S = [None] * NMT
            S[0] = stage_a(0, xt_pre=xt_first, xq=nc.scalar)
            b_murow(0, S[0])
            for c in range(FCH):
                b_chunk(0, S[0], c)
            S[1] = stage_a(1)
            b_murow(1, S[1])
            cg_interleaved(0, S[0], S[1])
            cbd_rest(0, S[0])
            S[2] = stage_a(2)
            b_murow(2, S[2])
            cg_interleaved(1, S[1], S[2])
            cbd_rest(1, S[1])
            S[3] = stage_a(3)
            b_murow(3, S[3])
            cg_interleaved(2, S[2], S[3])
            emit_bcast(2, S[2], 2)
            emit_bcast(2, S[2], 3)
            # tail: mt3 gate chain filled with mt2's M2 work
            cg_ss(3, S[3], 0)
            emit_m2(2, S[2], 0)
            cg_ss(3, S[3], 1)
            emit_m2(2, S[2], 1)
            cg_ss(3, S[3], 2)
            emit_m2(2, S[2], 2)
            cg_ss(3, S[3], 3)
            emit_m2(2, S[2], 3)
            cg_fin(3, S[3])
            emit_bcast(3, S[3], 0)
            emit_bcast(3, S[3], 1)
            cbd_rest(3, S[3])

    _split_multiwaits(nc)
    return nc


_built = {}


def _get_nc(has_b1e, has_b2, mode=None):
    key = (has_b1e, has_b2, mode or MODE)
    if key not in _built:
        _built[key] = _build(has_b1e, has_b2, mode)
    return _built[key]


last_results = None


def kernel(x, ln_g, ln_b, W1, b1, W2, b2, gu, gv, gb):
    import ml_dtypes
    E4 = ml_dtypes.float8_e4m3

    x = np.asarray(x, dtype=np.float32)
    ln_g = np.asarray(ln_g, dtype=np.float32)
    ln_b = np.asarray(ln_b, dtype=np.float32)
    W1 = np.asarray(W1, dtype=np.float32)
    b1 = np.asarray(b1, dtype=np.float32)
    W2 = np.asarray(W2, dtype=np.float32)
    b2 = np.asarray(b2, dtype=np.float32)
    gu = np.asarray(gu, dtype=np.float32)
    gv = np.asarray(gv, dtype=np.float32)
    gb = np.asarray(gb, dtype=np.float32)

    fp8 = MODE == "fp8"
    ndt = E4 if fp8 else np.float16
    SC = 8.0 if fp8 else 1.0

    # ---- host packing (weights/layout only) ----
    W1g = np.transpose(W1, (0, 2, 1)) * ln_g[:, :, None]       # [D,H,F]
    b1e = b1 + np.einsum('dfh,dh->df', W1, ln_b)               # [D,F]
    w2gv = np.einsum('dh,dhf->df', gv, W2)                     # [D,F]
    gb_eff = gb + np.einsum('dh,dh->d', gv, b2)                # [D]
    has_b1e = bool(np.any(b1e != 0.0))
    has_b2 = bool(np.any(b2 != 0.0))

    # M1 lhsT [128, KCH, DF], chunk c=(d, fh)
    w1full = np.zeros((128, KCH, DF), np.float32)
    for c in range(FCH):
        d, fh = c // 2, c % 2
        w1full[:, :, c * 128:(c + 1) * 128] = (
            SC * W1g[d].reshape(KCH, 128, F)[:, :, fh * 128:(fh + 1) * 128]
            .transpose(1, 0, 2))
    w1hi_in = w1full.astype(ndt)
    if fp8:
        w1lo_in = (w1full - w1hi_in.astype(np.float32)).astype(E4)
    # M2 rhs [128, FCH, H]: w2t[p, c, h] = W2[d, h, fh*128+p]
    w2t_in = np.zeros((128, FCH, H), np.float16)
    for c in range(FCH):
        d, fh = c // 2, c % 2
        w2t_in[:, c, :] = W2[d, :, fh * 128:(fh + 1) * 128].T
    # block-diag w2gv [128,18] + gu chunks [128,24] + ones8 [128,8]
    c16_in = np.zeros((128, 170), np.float16)
    for c in range(FCH):
        d, fh = c // 2, c % 2
        c16_in[:, c * D + d] = w2gv[d, fh * 128:(fh + 1) * 128]
    if not fp8:
        for k in range(KCH):
            c16_in[:, 18 + k * D:18 + (k + 1) * D] = \
                (SC * gu[:, k * 128:(k + 1) * 128]).T
    c16_in[:, 42:170] = 1.0
    row16_in = np.zeros((1, RO_END), np.float16)
    row16_in[0, RO_ONES:RO_ONES + 128] = 1.0
    row16_in[0, RO_GB:RO_GB + D] = SC * gb_eff
    row16_in[0, RO_W1S:RO_W1S + DF] = -w1full.sum(axis=(0, 1))
    if has_b1e:
        b1e_pack = np.zeros(DF, np.float32)
        for c in range(FCH):
            d, fh = c // 2, c % 2
            b1e_pack[c * 128:(c + 1) * 128] = \
                SC * b1e[d, fh * 128:(fh + 1) * 128]
        row16_in[0, RO_B1E:RO_B1E + DF] = b1e_pack
    oneh_in = np.zeros((12, 12 * 128), np.float16)
    for j in range(12):
        oneh_in[j, j * 128:(j + 1) * 128] = 1.0
    cpack_in = np.eye(128, dtype=np.float32)

    common = {
        "w1hi": w1hi_in, "w2t": w2t_in, "cpack": cpack_in,
        "cpk16": c16_in, "row16": row16_in, "oneh": oneh_in,
    }
    if fp8:
        common["w1lo"] = w1lo_in
        cpk8_in = np.zeros((128, 280), np.float32)
        for k in range(KCH):
            cpk8_in[:, k * D:(k + 1) * D] = \
                (SC * gu[:, k * 128:(k + 1) * 128]).T
        cpk8_in[:, 24:280] = 1.0
        common["cpk8"] = cpk8_in.astype(E4)
    if has_b2:
        common["b2bc"] = np.broadcast_to(
            b2[None, :, :], (128, D, H)).astype(np.float32).copy()

    nc = _get_nc(has_b1e, has_b2)

    in_maps = []
    for c in range(B):
        m = dict(common, xin=np.ascontiguousarray(x[c]))
        m["xt"] = np.ascontiguousarray(x[c].T).astype(ndt)
        in_maps.append(m)
    res = run_bass_kernel_spmd(nc, in_maps, core_ids=list(range(B)))
    global last_results
    last_results = res
    return np.stack([res.results[c]["out"] for c in range(B)])
